# revision 14
# baseline (speedup 1.0000x reference)
"""Trainium2 Bass kernel for nn_Attention_72438918414857.

Reference computation (B=8, N=1024, C=768, H=12, D=64):
    qkv = (x @ qkv_w.T + qkv_b) -> q, k, v per head
    attn = softmax(q @ k.T / sqrt(D)) + static_a   (bias added AFTER softmax)
    out = (attn @ v) merged-heads @ proj_w.T + proj_b

Sharding: data-parallel over batch -- one batch element per NeuronCore,
weights + static_a replicated. No collectives needed.

Math used on-chip (per batch, per head), everything transposed so each
matmul gets its contraction dim on partitions with no on-chip transposes:
    qkT = [Wq;Wk]^T-proj of x  ->  [cout, t] layout
    E^T = exp(K_h^T.T @ Q_h^T * D^-0.5)           [k, q] strips
    out_h^T = ([V_h|1].T @ E^T) -> rows 0..63 = E@v, row 64 = rowsum(E)
    attn_h^T = (E@v) * (1/rowsum) + V_h.T @ A_h^T
where static_a is pre-transposed on host to A^T[h, k, q].  The softmax
normalization is applied to the [64, q] output instead of the [k, q]
matrix; no max-subtraction is needed (|scores*scale| < ~3).

Matmuls run in bf16 (fp32 PE matmul is 4x slower); PSUM accumulation is
fp32.  bf16 rounding of operands keeps rel-err ~1e-3, well under the
2e-2 gate.
"""

import os
import sys

import numpy as np

B, N, C = 8, 1024, 768
H, D = 12, 64
NCORES = 8
P = 128
QW = 512          # q tile width (PSUM bank = 512 f32)
NQT = N // QW     # 2 q tiles
NKT = N // P      # 8 k tiles
NCIN = C // P     # 6 c_in chunks
NPAIR = H // 2    # 6 head pairs
SCALE = float(D) ** -0.5

_REPO = "/opt/trn_rl_repo"


def _ensure_paths():
    if _REPO not in sys.path:
        sys.path.insert(0, _REPO)


def _fuse_ldweights(nc):
    """Tile splits each matmul into Ldweights + Matmult (moving the input
    waits onto the Ldweights).  The Matmult still carries the weights
    operand, so the standalone Ldweights is droppable: delete it and move
    its waits/updates onto the matmul.  This makes every matmul
    self-loading, which walrus's LDW optimization (background weight
    buffer pipelining) requires."""
    import concourse.mybir as mybir

    for fn in nc.m.functions:
        for blk in fn.blocks:
            out = []
            pend_w, pend_u = [], []
            changed = False
            for inst in blk.instructions:
                op = str(inst.opcode)
                if op == "Ldweights":
                    si = inst.sync_info
                    if si:
                        pend_w.extend(si.on_wait or [])
                        pend_u.extend(si.on_update or [])
                    changed = True
                    continue
                if op == "Matmult" and (pend_w or pend_u):
                    si = inst.sync_info
                    ow = list(si.on_wait or []) if si else []
                    ou = list(si.on_update or []) if si else []
                    inst.sync_info = mybir.SyncInfo(
                        on_wait=pend_w + ow, on_update=pend_u + ou)
                    pend_w, pend_u = [], []
                out.append(inst)
            assert not pend_w and not pend_u, "dangling ldweights sync"
            if changed:
                blk.instructions = out


def _split_excess_waits(nc):
    """The TRN2 walrus codegen allows only 1 sem-wait command per
    instruction.  Tile's sem-assigner can emit more (one per logical
    proc a tile depends on).
    Move the excess onto freshly inserted same-engine NoOps placed just
    before the instruction -- engines execute in order, so waiting on a
    preceding NoOp is equivalent."""
    import concourse.mybir as mybir
    from bass_rust import InstNoOp

    nid = [0]
    for fn in nc.m.functions:
        for blk in fn.blocks:
            out = []
            changed = False
            for inst in blk.instructions:
                si = inst.sync_info
                waits = list(si.on_wait) if si and si.on_wait else []
                limit = 1
                if len(waits) > limit:
                    extra, keep = waits[:-limit], waits[-limit:]
                    inst.sync_info = si.__replace__(on_wait=keep)
                    for w in extra:
                        nop = InstNoOp(
                            name=f"{inst.name}-wsplit{nid[0]}", ins=[], outs=[])
                        nid[0] += 1
                        nop.engine = inst.engine
                        nop.sync_info = mybir.SyncInfo(
                            on_wait=[w], on_update=[])
                        out.append(nop)
                    changed = True
                out.append(inst)
            if changed:
                blk.instructions = out


def _patch_ldw_opt():
    """walrus ships with --enable-ldw-opt=false; enabling it lets the PE
    pipeline LDWEIGHTS with in-flight matmuls (background weight buffer),
    hiding the ~100ns weight-load per matmul."""
    from concourse import bass_utils
    if getattr(bass_utils.run_command, "_ldwopt", False):
        return
    orig = bass_utils.run_command

    def run_command_ldwopt(cmd, *a, **kw):
        cmd = [c.replace("--enable-ldw-opt=false", "--enable-ldw-opt=true")
               if isinstance(c, str) else c for c in cmd]
        return orig(cmd, *a, **kw)

    run_command_ldwopt._ldwopt = True
    bass_utils.run_command = run_command_ldwopt


def build_nc():
    """Build the per-core Bass/Tile program."""
    _ensure_paths()
    if os.environ.get("ATTN_LDW_OPT", "0") == "1":
        _patch_ldw_opt()
    import concourse.bass as bass
    import concourse.mybir as mybir
    import concourse.tile as tile
    from contextlib import ExitStack

    f32 = mybir.dt.float32
    bf16 = mybir.dt.bfloat16

    nc = bass.Bass("TRN2", target_bir_lowering=False, debug=False,
                   num_devices=NCORES)

    xT_ext = nc.declare_dram_parameter("xT", [C, N], f32, isOutput=False)
    qkwT_ext = nc.declare_dram_parameter("qkwT", [C, 2 * C], f32, isOutput=False)
    qkb_ext = nc.declare_dram_parameter("qkb", [P, 2 * C // P], f32, isOutput=False)
    vwT_ext = nc.declare_dram_parameter("vwT", [C, C], f32, isOutput=False)
    vb_ext = nc.declare_dram_parameter("vb", [1, C], f32, isOutput=False)
    at_ext = nc.declare_dram_parameter(
        "at", [NPAIR, NQT, NKT, P, 2 * QW], f32, isOutput=False)
    pwT_ext = nc.declare_dram_parameter("pwT", [C, C], f32, isOutput=False)
    pb_ext = nc.declare_dram_parameter("pb", [P, C // P], f32, isOutput=False)
    out_ext = nc.declare_dram_parameter("out", [C, N], f32, isOutput=True)

    NQK = 2 * C // P   # 12 cout tiles for q|k

    with tile.TileContext(nc, num_cores=NCORES) as tc, ExitStack() as ctx:
        consts = ctx.enter_context(tc.tile_pool(name="consts", bufs=1))
        persist = ctx.enter_context(tc.tile_pool(name="persist", bufs=1))
        attn_pool = ctx.enter_context(tc.tile_pool(name="attnout", bufs=1))
        epool = ctx.enter_context(tc.tile_pool(name="epool", bufs=2))
        at1 = ctx.enter_context(tc.tile_pool(name="at1", bufs=6))
        atbf1 = ctx.enter_context(tc.tile_pool(name="atbf1", bufs=4))
        small = ctx.enter_context(tc.tile_pool(name="small", bufs=3))

        qkb_sb = consts.tile([P, NQK], f32)
        nc.sync.dma_start(qkb_sb[:], qkb_ext[:])
        pb_sb = consts.tile([P, NCIN], f32)
        nc.sync.dma_start(pb_sb[:], pb_ext[:])
        vbf_sb = consts.tile([1, C], f32)
        nc.sync.dma_start(vbf_sb[:], vb_ext[:])
        vb_sb = consts.tile([1, C], bf16)
        nc.vector.tensor_copy(vb_sb[:], vbf_sb[:])
        ones_sb = consts.tile([1, P], bf16)
        nc.any.memset(ones_sb[:], 1.0)
        ones64_sb = consts.tile([1, 64], f32)
        nc.any.memset(ones64_sb[:], 1.0)

        # persistent activations (bf16 matmul operands)
        qkT_sb = persist.tile([P, NQK, N], bf16)      # [q|k]^T: cout x tokens
        vp_sb = persist.tile([P, H, NKT, 65], bf16)   # [V_h | 1] stationary
        nc.any.memset(vp_sb[:, :, :, 64:65], 1.0)
        pw_sb = persist.tile([P, NCIN, C], bf16)      # proj weights (bf16)
        attn_sb = attn_pool.tile([P, NCIN, N], bf16)  # attention out^T

        # ---------------- phase 1: qkv projections ----------------
        with tc.tile_pool(name="ph1", bufs=1) as ph1, \
             tc.tile_pool(name="stgp", bufs=2) as stgp, \
             tc.tile_pool(name="pp_qk", bufs=2, space="PSUM") as pp_qk, \
             tc.tile_pool(name="pp_v", bufs=2, space="PSUM") as pp_v:
            xT_sb = ph1.tile([P, NCIN, N], bf16)
            qkw_sb = ph1.tile([P, NCIN, 2 * C], bf16)
            vw_sb = ph1.tile([P, NCIN, C], bf16)
            # staged f32 loads (double-buffered, per-kchunk) casted into
            # bf16 tensors, so matmuls start before all weights land
            xT_r = xT_ext.rearrange("(c p) t -> p c t", p=P)
            qkw_r = qkwT_ext.rearrange("(c p) n -> p c n", p=P)
            vw_r = vwT_ext.rearrange("(c p) n -> p c n", p=P)
            pw_r = pwT_ext.rearrange("(c p) n -> p c n", p=P)
            loads = []
            for kc in range(NCIN):
                loads.append((xT_r[:, kc, :], xT_sb[:, kc, :], N))
            for kc in range(NCIN):
                loads.append((qkw_r[:, kc, :], qkw_sb[:, kc, :], 2 * C))
            for kc in range(NCIN):
                loads.append((vw_r[:, kc, :], vw_sb[:, kc, :], C))
            for kc in range(NCIN):
                loads.append((pw_r[:, kc, :], pw_sb[:, kc, :], C))
            for src_ap, dst_ap, w in loads:
                stg = stgp.tile([P, 2 * C], f32, tag="stage")
                nc.sync.dma_start(stg[:, 0:w], src_ap)
                nc.vector.tensor_copy(dst_ap, stg[:, 0:w])

            # qkT[ct] = qkwT_slice.T @ xT  (+ per-partition bias, cast bf16)
            for ct in range(NQK):
                ps = pp_qk.tile([P, N], f32, tag="qk")
                for qh in range(NQT):
                    for kc in range(NCIN):
                        nc.tensor.matmul(
                            ps[:, qh * QW:(qh + 1) * QW],
                            qkw_sb[:, kc, ct * P:(ct + 1) * P],
                            xT_sb[:, kc, qh * QW:(qh + 1) * QW],
                            start=(kc == 0), stop=(kc == NCIN - 1))
                nc.vector.tensor_scalar_add(
                    qkT_sb[:, ct, :], ps[:, :], qkb_sb[:, ct:ct + 1])

            # V[tt] = xT_slice.T @ vwT (+ ones x vb rank-1 bias), cast bf16
            for tt in range(NKT):
                ps = pp_v.tile([P, C], f32, tag="v")
                for (n0, nw) in ((0, QW), (QW, C - QW)):
                    for kc in range(NCIN):
                        nc.tensor.matmul(
                            ps[:, n0:n0 + nw],
                            xT_sb[:, kc, tt * P:(tt + 1) * P],
                            vw_sb[:, kc, n0:n0 + nw],
                            start=(kc == 0), stop=False)
                    nc.tensor.matmul(
                        ps[:, n0:n0 + nw],
                        ones_sb[0:1, 0:P],
                        vb_sb[0:1, n0:n0 + nw],
                        start=False, stop=True)
                nc.vector.tensor_copy(
                    vp_sb[:, :, tt, 0:64],
                    ps.rearrange("p (h d) -> p h d", d=64))

        # ---------------- phase 2: attention ----------------
        with tc.tile_pool(name="at2", bufs=10) as at2, \
             tc.tile_pool(name="atbf2", bufs=7) as atbf2, \
             tc.tile_pool(name="pp_st", bufs=2, space="PSUM") as pp_st, \
             tc.tile_pool(name="pp_ev", bufs=2, space="PSUM") as pp_ev, \
             tc.tile_pool(name="pp_av", bufs=1, space="PSUM") as pp_av, \
             tc.tile_pool(name="pp_r", bufs=1, space="PSUM") as pp_r:
            for pr in range(NPAIR):
                h1, h2 = 2 * pr, 2 * pr + 1
                for qt in range(NQT):
                    q0 = qt * QW
                    # --- scores + exp: E^T strips [k, q] for both heads ---
                    e_sb = epool.tile([P, NKT, 2 * QW], bf16, tag="e")
                    for kt in range(NKT):
                        st = pp_st.tile([P, 2 * QW], f32, tag="st")
                        k0 = kt * P
                        nc.tensor.matmul(
                            st[:, 0:QW],
                            qkT_sb[0:64, NPAIR + pr, k0:k0 + P],
                            qkT_sb[0:64, pr, q0:q0 + QW],
                            start=True, stop=True)
                        nc.tensor.matmul(
                            st[:, QW:2 * QW],
                            qkT_sb[64:128, NPAIR + pr, k0:k0 + P],
                            qkT_sb[64:128, pr, q0:q0 + QW],
                            start=True, stop=True)
                        nc.scalar.activation(
                            e_sb[:, kt, :], st[:, :],
                            mybir.ActivationFunctionType.Exp, scale=SCALE)

                    # --- E@v (+rowsum via ones col) and A@v ---
                    psE1 = pp_ev.tile([65, QW], f32, tag="ev")
                    psE2 = pp_ev.tile([65, QW], f32, tag="ev")
                    psA = pp_av.tile([P, QW], f32, tag="av")
                    for kt in range(NKT):
                        ap_f = at1 if pr == 0 else at2
                        ap_b = atbf1 if pr == 0 else atbf2
                        at_f = ap_f.tile([P, 2 * QW], f32, tag="at")
                        nc.sync.dma_start(at_f[:], at_ext[pr, qt, kt])
                        at = ap_b.tile([P, 2 * QW], bf16, tag="atb")
                        nc.vector.tensor_copy(at[:], at_f[:])
                        st_flags = dict(start=(kt == 0), stop=(kt == NKT - 1))
                        nc.tensor.matmul(
                            psE1[:, :], vp_sb[:, h1, kt, :],
                            e_sb[:, kt, 0:QW], **st_flags)
                        nc.tensor.matmul(
                            psE2[:, :], vp_sb[:, h2, kt, :],
                            e_sb[:, kt, QW:2 * QW], **st_flags)
                        nc.tensor.matmul(
                            psA[0:64, :], vp_sb[:, h1, kt, 0:64],
                            at[:, 0:QW], **st_flags)
                        nc.tensor.matmul(
                            psA[64:128, :], vp_sb[:, h2, kt, 0:64],
                            at[:, QW:2 * QW], **st_flags)

                    # --- epilogue: out_h = E@v * (1/rowsum) + A@v ---
                    for hi, psE in ((0, psE1), (1, psE2)):
                        pa, pz = hi * 64, hi * 64 + 64
                        lns_sb = small.tile([1, QW], f32, tag="lns")
                        nc.scalar.activation(
                            lns_sb[:], psE[64:65, :],
                            mybir.ActivationFunctionType.Ln)
                        r_sb = small.tile([1, QW], f32, tag="r")
                        nc.scalar.activation(
                            r_sb[:], lns_sb[:],
                            mybir.ActivationFunctionType.Exp, scale=-1.0)
                        psR = pp_r.tile([64, QW], f32, tag="rp")
                        nc.tensor.matmul(psR[:, :], ones64_sb[:, :], r_sb[:, :],
                                         start=True, stop=True)
                        rb_sb = small.tile([64, QW], f32, tag="rb")
                        nc.vector.tensor_copy(rb_sb[:], psR[:, :])
                        dst = attn_sb[pa:pz, pr, q0:q0 + QW]
                        nc.vector.tensor_mul(dst, psE[0:64, :], rb_sb[:])
                        nc.vector.tensor_add(dst, dst, psA[pa:pz, :])

        # ---------------- phase 3: output projection ----------------
        with tc.tile_pool(name="ph3o", bufs=2) as ph3o, \
             tc.tile_pool(name="pp_p", bufs=2, space="PSUM") as pp_p:
            out_r = out_ext.rearrange("(c p) t -> p c t", p=P)
            for ct in range(NCIN):
                ps = pp_p.tile([P, N], f32, tag="pp")
                for qh in range(NQT):
                    for kc in range(NCIN):
                        nc.tensor.matmul(
                            ps[:, qh * QW:(qh + 1) * QW],
                            pw_sb[:, kc, ct * P:(ct + 1) * P],
                            attn_sb[:, kc, qh * QW:(qh + 1) * QW],
                            start=(kc == 0), stop=(kc == NCIN - 1))
                o_sb = ph3o.tile([P, N], f32, tag="o")
                nc.vector.tensor_scalar_add(o_sb[:], ps[:], pb_sb[:, ct:ct + 1])
                nc.sync.dma_start(out_r[:, ct, :], o_sb[:])

    if os.environ.get("ATTN_FUSE_LDW", "0") == "1":
        _fuse_ldweights(nc)
    _split_excess_waits(nc)
    return nc


def make_in_maps(x, qkv_w, qkv_b, static_a, proj_w, proj_b):
    """Host-side sharding / layout prep. One batch element per core."""
    x = np.asarray(x, dtype=np.float32)
    qkv_w = np.asarray(qkv_w, dtype=np.float32)
    qkv_b = np.asarray(qkv_b, dtype=np.float32)
    static_a = np.asarray(static_a, dtype=np.float32)
    proj_w = np.asarray(proj_w, dtype=np.float32)
    proj_b = np.asarray(proj_b, dtype=np.float32)

    qkwT = np.ascontiguousarray(qkv_w[0:2 * C].T)            # [768, 1536]
    qkb = np.ascontiguousarray(qkv_b[0:2 * C].reshape(2 * C // P, P).T)
    vwT = np.ascontiguousarray(qkv_w[2 * C:3 * C].T)         # [768, 768]
    vb = np.ascontiguousarray(qkv_b[2 * C:3 * C].reshape(1, C))
    # A^T strips, contiguous per (pair, qtile, ktile): [6, 2, 8, 128, 1024]
    # at[pr, qt, kt, :, 0:512] = A^T[2pr][kt tile, qt tile], [..., 512:] = head 2pr+1
    atT = static_a[0].transpose(0, 2, 1)                      # [H, k, q]
    at = np.ascontiguousarray(
        atT.reshape(NPAIR, 2, NKT, P, NQT, QW).transpose(0, 4, 2, 3, 1, 5)
        .reshape(NPAIR, NQT, NKT, P, 2 * QW))
    pwT = np.ascontiguousarray(proj_w.T)
    pb = np.ascontiguousarray(proj_b.reshape(C // P, P).T)

    shared = {"qkwT": qkwT, "qkb": qkb, "vwT": vwT, "vb": vb,
              "at": at, "pwT": pwT, "pb": pb}
    in_maps = []
    for b in range(B):
        m = dict(shared)
        m["xT"] = np.ascontiguousarray(x[b].T)
        in_maps.append(m)
    return in_maps


_NC_CACHE = {}


def _get_nc():
    if "nc" not in _NC_CACHE:
        _NC_CACHE["nc"] = build_nc()
    return _NC_CACHE["nc"]


def kernel(x, qkv_w, qkv_b, static_a, proj_w, proj_b):
    _ensure_paths()
    from concourse.bass_utils import run_bass_kernel_spmd

    nc = _get_nc()
    in_maps = make_in_maps(x, qkv_w, qkv_b, static_a, proj_w, proj_b)
    res = run_bass_kernel_spmd(nc, in_maps, core_ids=list(range(NCORES)))
    out = np.empty((B, N, C), dtype=np.float32)
    for b in range(B):
        out[b] = res.results[b]["out"].T
    return out


# revision 15
# speedup vs baseline: 1.0101x; 1.0101x over previous
"""Trainium2 Bass kernel for nn_Attention_72438918414857.

Reference computation (B=8, N=1024, C=768, H=12, D=64):
    qkv = (x @ qkv_w.T + qkv_b) -> q, k, v per head
    attn = softmax(q @ k.T / sqrt(D)) + static_a   (bias added AFTER softmax)
    out = (attn @ v) merged-heads @ proj_w.T + proj_b

Sharding: data-parallel over batch -- one batch element per NeuronCore,
weights + static_a replicated. No collectives needed.

Math used on-chip (per batch, per head), everything transposed so each
matmul gets its contraction dim on partitions with no on-chip transposes:
    qkT = [Wq;Wk]^T-proj of x  ->  [cout, t] layout
    E^T = exp(K_h^T.T @ Q_h^T * D^-0.5)           [k, q] strips
    out_h^T = ([V_h|1].T @ E^T) -> rows 0..63 = E@v, row 64 = rowsum(E)
    attn_h^T = (E@v) * (1/rowsum) + V_h.T @ A_h^T
where static_a is pre-transposed on host to A^T[h, k, q].  The softmax
normalization is applied to the [64, q] output instead of the [k, q]
matrix; no max-subtraction is needed (|scores*scale| < ~3).

Matmuls run in bf16 (fp32 PE matmul is 4x slower); PSUM accumulation is
fp32.  bf16 rounding of operands keeps rel-err ~1e-3, well under the
2e-2 gate.
"""

import os
import sys

import numpy as np

B, N, C = 8, 1024, 768
H, D = 12, 64
NCORES = 8
P = 128
QW = 512          # q tile width (PSUM bank = 512 f32)
NQT = N // QW     # 2 q tiles
NKT = N // P      # 8 k tiles
NCIN = C // P     # 6 c_in chunks
NPAIR = H // 2    # 6 head pairs
SCALE = float(D) ** -0.5

_REPO = "/opt/trn_rl_repo"


def _ensure_paths():
    if _REPO not in sys.path:
        sys.path.insert(0, _REPO)


def _fuse_ldweights(nc):
    """Tile splits each matmul into Ldweights + Matmult (moving the input
    waits onto the Ldweights).  The Matmult still carries the weights
    operand, so the standalone Ldweights is droppable: delete it and move
    its waits/updates onto the matmul.  This makes every matmul
    self-loading, which walrus's LDW optimization (background weight
    buffer pipelining) requires."""
    import concourse.mybir as mybir

    for fn in nc.m.functions:
        for blk in fn.blocks:
            out = []
            pend_w, pend_u = [], []
            changed = False
            for inst in blk.instructions:
                op = str(inst.opcode)
                if op == "Ldweights":
                    si = inst.sync_info
                    if si:
                        pend_w.extend(si.on_wait or [])
                        pend_u.extend(si.on_update or [])
                    changed = True
                    continue
                if op == "Matmult" and (pend_w or pend_u):
                    si = inst.sync_info
                    ow = list(si.on_wait or []) if si else []
                    ou = list(si.on_update or []) if si else []
                    inst.sync_info = mybir.SyncInfo(
                        on_wait=pend_w + ow, on_update=pend_u + ou)
                    pend_w, pend_u = [], []
                out.append(inst)
            assert not pend_w and not pend_u, "dangling ldweights sync"
            if changed:
                blk.instructions = out


def _split_excess_waits(nc):
    """The TRN2 walrus codegen allows only 1 sem-wait command per
    instruction.  Tile's sem-assigner can emit more (one per logical
    proc a tile depends on).
    Move the excess onto freshly inserted same-engine NoOps placed just
    before the instruction -- engines execute in order, so waiting on a
    preceding NoOp is equivalent."""
    import concourse.mybir as mybir
    from bass_rust import InstNoOp

    nid = [0]
    for fn in nc.m.functions:
        for blk in fn.blocks:
            out = []
            changed = False
            for inst in blk.instructions:
                si = inst.sync_info
                waits = list(si.on_wait) if si and si.on_wait else []
                limit = 1
                if len(waits) > limit:
                    extra, keep = waits[:-limit], waits[-limit:]
                    inst.sync_info = si.__replace__(on_wait=keep)
                    for w in extra:
                        nop = InstNoOp(
                            name=f"{inst.name}-wsplit{nid[0]}", ins=[], outs=[])
                        nid[0] += 1
                        nop.engine = inst.engine
                        nop.sync_info = mybir.SyncInfo(
                            on_wait=[w], on_update=[])
                        out.append(nop)
                    changed = True
                out.append(inst)
            if changed:
                blk.instructions = out


def _patch_ldw_opt():
    """walrus ships with --enable-ldw-opt=false; enabling it lets the PE
    pipeline LDWEIGHTS with in-flight matmuls (background weight buffer),
    hiding the ~100ns weight-load per matmul."""
    from concourse import bass_utils
    if getattr(bass_utils.run_command, "_ldwopt", False):
        return
    orig = bass_utils.run_command

    def run_command_ldwopt(cmd, *a, **kw):
        cmd = [c.replace("--enable-ldw-opt=false", "--enable-ldw-opt=true")
               if isinstance(c, str) else c for c in cmd]
        return orig(cmd, *a, **kw)

    run_command_ldwopt._ldwopt = True
    bass_utils.run_command = run_command_ldwopt


def build_nc():
    """Build the per-core Bass/Tile program."""
    _ensure_paths()
    if os.environ.get("ATTN_LDW_OPT", "0") == "1":
        _patch_ldw_opt()
    import concourse.bass as bass
    import concourse.mybir as mybir
    import concourse.tile as tile
    from contextlib import ExitStack

    f32 = mybir.dt.float32
    bf16 = mybir.dt.bfloat16

    nc = bass.Bass("TRN2", target_bir_lowering=False, debug=False,
                   num_devices=NCORES)

    xT_ext = nc.declare_dram_parameter("xT", [C, N], f32, isOutput=False)
    qkwT_ext = nc.declare_dram_parameter("qkwT", [C, 2 * C], f32, isOutput=False)
    qkb_ext = nc.declare_dram_parameter("qkb", [P, 2 * C // P], f32, isOutput=False)
    vwT_ext = nc.declare_dram_parameter("vwT", [C, C], f32, isOutput=False)
    vb_ext = nc.declare_dram_parameter("vb", [1, C], f32, isOutput=False)
    at_ext = nc.declare_dram_parameter(
        "at", [NPAIR, NQT, NKT, P, 2 * QW], f32, isOutput=False)
    pwT_ext = nc.declare_dram_parameter("pwT", [C, C], f32, isOutput=False)
    pb_ext = nc.declare_dram_parameter("pb", [P, C // P], f32, isOutput=False)
    out_ext = nc.declare_dram_parameter("out", [C, N], f32, isOutput=True)

    NQK = 2 * C // P   # 12 cout tiles for q|k

    with tile.TileContext(nc, num_cores=NCORES) as tc, ExitStack() as ctx:
        consts = ctx.enter_context(tc.tile_pool(name="consts", bufs=1))
        persist = ctx.enter_context(tc.tile_pool(name="persist", bufs=1))
        attn_pool = ctx.enter_context(tc.tile_pool(name="attnout", bufs=1))
        epool = ctx.enter_context(tc.tile_pool(name="epool", bufs=2))
        at1 = ctx.enter_context(tc.tile_pool(name="at1", bufs=6))
        atbf1 = ctx.enter_context(tc.tile_pool(name="atbf1", bufs=4))
        small = ctx.enter_context(tc.tile_pool(name="small", bufs=3))

        qkb_sb = consts.tile([P, NQK], f32)
        nc.sync.dma_start(qkb_sb[:], qkb_ext[:])
        pb_sb = consts.tile([P, NCIN], f32)
        nc.sync.dma_start(pb_sb[:], pb_ext[:])
        vbf_sb = consts.tile([1, C], f32)
        nc.sync.dma_start(vbf_sb[:], vb_ext[:])
        vb_sb = consts.tile([1, C], bf16)
        nc.vector.tensor_copy(vb_sb[:], vbf_sb[:])
        ones_sb = consts.tile([1, P], bf16)
        nc.any.memset(ones_sb[:], 1.0)
        ones64_sb = consts.tile([1, 64], f32)
        nc.any.memset(ones64_sb[:], 1.0)

        # persistent activations (bf16 matmul operands)
        qkT_sb = persist.tile([P, NQK, N], bf16)      # [q|k]^T: cout x tokens
        vp_sb = persist.tile([P, H, NKT, 65], bf16)   # [V_h | 1] stationary
        nc.any.memset(vp_sb[:, :, :, 64:65], 1.0)
        pw_sb = persist.tile([P, NCIN, C], bf16)      # proj weights (bf16)
        attn_sb = attn_pool.tile([P, NCIN, N], bf16)  # attention out^T

        # ---------------- phase 1: qkv projections ----------------
        with tc.tile_pool(name="ph1", bufs=1) as ph1, \
             tc.tile_pool(name="stgp", bufs=2) as stgp, \
             tc.tile_pool(name="pp_qk", bufs=2, space="PSUM") as pp_qk, \
             tc.tile_pool(name="pp_v", bufs=2, space="PSUM") as pp_v:
            xT_sb = ph1.tile([P, NCIN, N], bf16)
            qkw_sb = ph1.tile([P, NCIN, 2 * C], bf16)
            vw_sb = ph1.tile([P, NCIN, C], bf16)
            # staged f32 loads (double-buffered, per-kchunk) casted into
            # bf16 tensors, so matmuls start before all weights land
            xT_r = xT_ext.rearrange("(c p) t -> p c t", p=P)
            qkw_r = qkwT_ext.rearrange("(c p) n -> p c n", p=P)
            vw_r = vwT_ext.rearrange("(c p) n -> p c n", p=P)
            pw_r = pwT_ext.rearrange("(c p) n -> p c n", p=P)
            loads = []
            for kc in range(NCIN):
                loads.append((xT_r[:, kc, :], xT_sb[:, kc, :], N))
                loads.append((vw_r[:, kc, :], vw_sb[:, kc, :], C))
            for kc in range(NCIN):
                loads.append((qkw_r[:, kc, :], qkw_sb[:, kc, :], 2 * C))
            for kc in range(NCIN):
                loads.append((pw_r[:, kc, :], pw_sb[:, kc, :], C))
            for src_ap, dst_ap, w in loads:
                stg = stgp.tile([P, 2 * C], f32, tag="stage")
                nc.sync.dma_start(stg[:, 0:w], src_ap)
                nc.vector.tensor_copy(dst_ap, stg[:, 0:w])

            # V[tt] = xT_slice.T @ vwT (+ ones x vb rank-1 bias), cast bf16
            for tt in range(NKT):
                ps = pp_v.tile([P, C], f32, tag="v")
                for (n0, nw) in ((0, QW), (QW, C - QW)):
                    for kc in range(NCIN):
                        nc.tensor.matmul(
                            ps[:, n0:n0 + nw],
                            xT_sb[:, kc, tt * P:(tt + 1) * P],
                            vw_sb[:, kc, n0:n0 + nw],
                            start=(kc == 0), stop=False)
                    nc.tensor.matmul(
                        ps[:, n0:n0 + nw],
                        ones_sb[0:1, 0:P],
                        vb_sb[0:1, n0:n0 + nw],
                        start=False, stop=True)
                nc.vector.tensor_copy(
                    vp_sb[:, :, tt, 0:64],
                    ps.rearrange("p (h d) -> p h d", d=64))

            # qkT[ct] = qkwT_slice.T @ xT  (+ per-partition bias, cast bf16)
            for ct in range(NQK):
                ps = pp_qk.tile([P, N], f32, tag="qk")
                for qh in range(NQT):
                    for kc in range(NCIN):
                        nc.tensor.matmul(
                            ps[:, qh * QW:(qh + 1) * QW],
                            qkw_sb[:, kc, ct * P:(ct + 1) * P],
                            xT_sb[:, kc, qh * QW:(qh + 1) * QW],
                            start=(kc == 0), stop=(kc == NCIN - 1))
                nc.vector.tensor_scalar_add(
                    qkT_sb[:, ct, :], ps[:, :], qkb_sb[:, ct:ct + 1])

        # ---------------- phase 2: attention ----------------
        with tc.tile_pool(name="at2", bufs=10) as at2, \
             tc.tile_pool(name="atbf2", bufs=7) as atbf2, \
             tc.tile_pool(name="pp_st", bufs=2, space="PSUM") as pp_st, \
             tc.tile_pool(name="pp_ev", bufs=2, space="PSUM") as pp_ev, \
             tc.tile_pool(name="pp_av", bufs=1, space="PSUM") as pp_av, \
             tc.tile_pool(name="pp_r", bufs=1, space="PSUM") as pp_r:
            for pr in range(NPAIR):
                h1, h2 = 2 * pr, 2 * pr + 1
                for qt in range(NQT):
                    q0 = qt * QW
                    # --- scores + exp: E^T strips [k, q] for both heads ---
                    e_sb = epool.tile([P, NKT, 2 * QW], bf16, tag="e")
                    for kt in range(NKT):
                        st = pp_st.tile([P, 2 * QW], f32, tag="st")
                        k0 = kt * P
                        nc.tensor.matmul(
                            st[:, 0:QW],
                            qkT_sb[0:64, NPAIR + pr, k0:k0 + P],
                            qkT_sb[0:64, pr, q0:q0 + QW],
                            start=True, stop=True)
                        nc.tensor.matmul(
                            st[:, QW:2 * QW],
                            qkT_sb[64:128, NPAIR + pr, k0:k0 + P],
                            qkT_sb[64:128, pr, q0:q0 + QW],
                            start=True, stop=True)
                        nc.scalar.activation(
                            e_sb[:, kt, :], st[:, :],
                            mybir.ActivationFunctionType.Exp, scale=SCALE)

                    # --- E@v (+rowsum via ones col) and A@v ---
                    psE1 = pp_ev.tile([65, QW], f32, tag="ev")
                    psE2 = pp_ev.tile([65, QW], f32, tag="ev")
                    psA = pp_av.tile([P, QW], f32, tag="av")
                    for kt in range(NKT):
                        ap_f = at1 if pr == 0 else at2
                        ap_b = atbf1 if pr == 0 else atbf2
                        at_f = ap_f.tile([P, 2 * QW], f32, tag="at")
                        nc.sync.dma_start(at_f[:], at_ext[pr, qt, kt])
                        at = ap_b.tile([P, 2 * QW], bf16, tag="atb")
                        nc.vector.tensor_copy(at[:], at_f[:])
                        st_flags = dict(start=(kt == 0), stop=(kt == NKT - 1))
                        nc.tensor.matmul(
                            psE1[:, :], vp_sb[:, h1, kt, :],
                            e_sb[:, kt, 0:QW], **st_flags)
                        nc.tensor.matmul(
                            psE2[:, :], vp_sb[:, h2, kt, :],
                            e_sb[:, kt, QW:2 * QW], **st_flags)
                        nc.tensor.matmul(
                            psA[0:64, :], vp_sb[:, h1, kt, 0:64],
                            at[:, 0:QW], **st_flags)
                        nc.tensor.matmul(
                            psA[64:128, :], vp_sb[:, h2, kt, 0:64],
                            at[:, QW:2 * QW], **st_flags)

                    # --- epilogue: out_h = E@v * (1/rowsum) + A@v ---
                    for hi, psE in ((0, psE1), (1, psE2)):
                        pa, pz = hi * 64, hi * 64 + 64
                        lns_sb = small.tile([1, QW], f32, tag="lns")
                        nc.scalar.activation(
                            lns_sb[:], psE[64:65, :],
                            mybir.ActivationFunctionType.Ln)
                        r_sb = small.tile([1, QW], f32, tag="r")
                        nc.scalar.activation(
                            r_sb[:], lns_sb[:],
                            mybir.ActivationFunctionType.Exp, scale=-1.0)
                        psR = pp_r.tile([64, QW], f32, tag="rp")
                        nc.tensor.matmul(psR[:, :], ones64_sb[:, :], r_sb[:, :],
                                         start=True, stop=True)
                        rb_sb = small.tile([64, QW], f32, tag="rb")
                        nc.vector.tensor_copy(rb_sb[:], psR[:, :])
                        dst = attn_sb[pa:pz, pr, q0:q0 + QW]
                        nc.vector.tensor_mul(dst, psE[0:64, :], rb_sb[:])
                        nc.vector.tensor_add(dst, dst, psA[pa:pz, :])

        # ---------------- phase 3: output projection ----------------
        with tc.tile_pool(name="ph3o", bufs=2) as ph3o, \
             tc.tile_pool(name="pp_p", bufs=2, space="PSUM") as pp_p:
            out_r = out_ext.rearrange("(c p) t -> p c t", p=P)
            for ct in range(NCIN):
                ps = pp_p.tile([P, N], f32, tag="pp")
                for qh in range(NQT):
                    for kc in range(NCIN):
                        nc.tensor.matmul(
                            ps[:, qh * QW:(qh + 1) * QW],
                            pw_sb[:, kc, ct * P:(ct + 1) * P],
                            attn_sb[:, kc, qh * QW:(qh + 1) * QW],
                            start=(kc == 0), stop=(kc == NCIN - 1))
                o_sb = ph3o.tile([P, N], f32, tag="o")
                nc.vector.tensor_scalar_add(o_sb[:], ps[:], pb_sb[:, ct:ct + 1])
                nc.sync.dma_start(out_r[:, ct, :], o_sb[:])

    if os.environ.get("ATTN_FUSE_LDW", "0") == "1":
        _fuse_ldweights(nc)
    _split_excess_waits(nc)
    return nc


def make_in_maps(x, qkv_w, qkv_b, static_a, proj_w, proj_b):
    """Host-side sharding / layout prep. One batch element per core."""
    x = np.asarray(x, dtype=np.float32)
    qkv_w = np.asarray(qkv_w, dtype=np.float32)
    qkv_b = np.asarray(qkv_b, dtype=np.float32)
    static_a = np.asarray(static_a, dtype=np.float32)
    proj_w = np.asarray(proj_w, dtype=np.float32)
    proj_b = np.asarray(proj_b, dtype=np.float32)

    qkwT = np.ascontiguousarray(qkv_w[0:2 * C].T)            # [768, 1536]
    qkb = np.ascontiguousarray(qkv_b[0:2 * C].reshape(2 * C // P, P).T)
    vwT = np.ascontiguousarray(qkv_w[2 * C:3 * C].T)         # [768, 768]
    vb = np.ascontiguousarray(qkv_b[2 * C:3 * C].reshape(1, C))
    # A^T strips, contiguous per (pair, qtile, ktile): [6, 2, 8, 128, 1024]
    # at[pr, qt, kt, :, 0:512] = A^T[2pr][kt tile, qt tile], [..., 512:] = head 2pr+1
    atT = static_a[0].transpose(0, 2, 1)                      # [H, k, q]
    at = np.ascontiguousarray(
        atT.reshape(NPAIR, 2, NKT, P, NQT, QW).transpose(0, 4, 2, 3, 1, 5)
        .reshape(NPAIR, NQT, NKT, P, 2 * QW))
    pwT = np.ascontiguousarray(proj_w.T)
    pb = np.ascontiguousarray(proj_b.reshape(C // P, P).T)

    shared = {"qkwT": qkwT, "qkb": qkb, "vwT": vwT, "vb": vb,
              "at": at, "pwT": pwT, "pb": pb}
    in_maps = []
    for b in range(B):
        m = dict(shared)
        m["xT"] = np.ascontiguousarray(x[b].T)
        in_maps.append(m)
    return in_maps


_NC_CACHE = {}


def _get_nc():
    if "nc" not in _NC_CACHE:
        _NC_CACHE["nc"] = build_nc()
    return _NC_CACHE["nc"]


def kernel(x, qkv_w, qkv_b, static_a, proj_w, proj_b):
    _ensure_paths()
    from concourse.bass_utils import run_bass_kernel_spmd

    nc = _get_nc()
    in_maps = make_in_maps(x, qkv_w, qkv_b, static_a, proj_w, proj_b)
    res = run_bass_kernel_spmd(nc, in_maps, core_ids=list(range(NCORES)))
    out = np.empty((B, N, C), dtype=np.float32)
    for b in range(B):
        out[b] = res.results[b]["out"].T
    return out


# revision 16
# speedup vs baseline: 1.0348x; 1.0244x over previous
"""Trainium2 Bass kernel for nn_Attention_72438918414857.

Reference computation (B=8, N=1024, C=768, H=12, D=64):
    qkv = (x @ qkv_w.T + qkv_b) -> q, k, v per head
    attn = softmax(q @ k.T / sqrt(D)) + static_a   (bias added AFTER softmax)
    out = (attn @ v) merged-heads @ proj_w.T + proj_b

Sharding: data-parallel over batch -- one batch element per NeuronCore,
weights + static_a replicated. No collectives needed.

Math used on-chip (per batch, per head), everything transposed so each
matmul gets its contraction dim on partitions with no on-chip transposes:
    qkT = [Wq;Wk]^T-proj of x  ->  [cout, t] layout
    E^T = exp(K_h^T.T @ Q_h^T * D^-0.5)           [k, q] strips
    out_h^T = ([V_h|1].T @ E^T) -> rows 0..63 = E@v, row 64 = rowsum(E)
    attn_h^T = (E@v) * (1/rowsum) + V_h.T @ A_h^T
where static_a is pre-transposed on host to A^T[h, k, q].  The softmax
normalization is applied to the [64, q] output instead of the [k, q]
matrix; no max-subtraction is needed (|scores*scale| < ~3).

Matmuls run in bf16 (fp32 PE matmul is 4x slower); PSUM accumulation is
fp32.  bf16 rounding of operands keeps rel-err ~1e-3, well under the
2e-2 gate.
"""

import os
import sys

import numpy as np

B, N, C = 8, 1024, 768
H, D = 12, 64
NCORES = 8
P = 128
QW = 512          # q tile width (PSUM bank = 512 f32)
NQT = N // QW     # 2 q tiles
NKT = N // P      # 8 k tiles
NCIN = C // P     # 6 c_in chunks
NPAIR = H // 2    # 6 head pairs
SCALE = float(D) ** -0.5

_REPO = "/opt/trn_rl_repo"


def _ensure_paths():
    if _REPO not in sys.path:
        sys.path.insert(0, _REPO)


def _fuse_ldweights(nc):
    """Tile splits each matmul into Ldweights + Matmult (moving the input
    waits onto the Ldweights).  The Matmult still carries the weights
    operand, so the standalone Ldweights is droppable: delete it and move
    its waits/updates onto the matmul.  This makes every matmul
    self-loading, which walrus's LDW optimization (background weight
    buffer pipelining) requires."""
    import concourse.mybir as mybir

    for fn in nc.m.functions:
        for blk in fn.blocks:
            out = []
            pend_w, pend_u = [], []
            changed = False
            for inst in blk.instructions:
                op = str(inst.opcode)
                if op == "Ldweights":
                    si = inst.sync_info
                    if si:
                        pend_w.extend(si.on_wait or [])
                        pend_u.extend(si.on_update or [])
                    changed = True
                    continue
                if op == "Matmult" and (pend_w or pend_u):
                    si = inst.sync_info
                    ow = list(si.on_wait or []) if si else []
                    ou = list(si.on_update or []) if si else []
                    inst.sync_info = mybir.SyncInfo(
                        on_wait=pend_w + ow, on_update=pend_u + ou)
                    pend_w, pend_u = [], []
                out.append(inst)
            assert not pend_w and not pend_u, "dangling ldweights sync"
            if changed:
                blk.instructions = out


def _dedup_ldweights(nc):
    """Delete an Ldweights whose weights AP + tile geometry match the
    immediately preceding Ldweights on the PE stream (the weights are
    still resident in the array); its waits/updates move to the next
    instruction."""
    import concourse.mybir as mybir

    def sig(inst):
        ap = inst.ins[0]
        return (str(ap), str(getattr(inst, "tile_position", None)),
                str(getattr(inst, "tile_size", None)))

    for fn in nc.m.functions:
        for blk in fn.blocks:
            out = []
            last_sig = None
            pend_w, pend_u = [], []
            changed = False
            for inst in blk.instructions:
                op = str(inst.opcode)
                if op == "Ldweights":
                    s_ = sig(inst)
                    if s_ == last_sig:
                        si = inst.sync_info
                        if si:
                            pend_w.extend(si.on_wait or [])
                            pend_u.extend(si.on_update or [])
                        changed = True
                        continue
                    last_sig = s_
                elif op == "Matmult":
                    pass          # matmuls don't disturb loaded weights
                elif op in ("NoOp", "EventSemaphore"):
                    pass
                else:
                    last_sig = None
                if pend_w or pend_u:
                    si = inst.sync_info
                    ow = list(si.on_wait or []) if si else []
                    ou = list(si.on_update or []) if si else []
                    inst.sync_info = mybir.SyncInfo(
                        on_wait=pend_w + ow, on_update=pend_u + ou)
                    pend_w, pend_u = [], []
                out.append(inst)
            assert not pend_w and not pend_u
            if changed:
                blk.instructions = out


def _split_excess_waits(nc):
    """The TRN2 walrus codegen allows only 1 sem-wait command per
    instruction.  Tile's sem-assigner can emit more (one per logical
    proc a tile depends on).
    Move the excess onto freshly inserted same-engine NoOps placed just
    before the instruction -- engines execute in order, so waiting on a
    preceding NoOp is equivalent."""
    import concourse.mybir as mybir
    from bass_rust import InstNoOp

    nid = [0]
    for fn in nc.m.functions:
        for blk in fn.blocks:
            out = []
            changed = False
            for inst in blk.instructions:
                si = inst.sync_info
                waits = list(si.on_wait) if si and si.on_wait else []
                limit = 1
                if len(waits) > limit:
                    extra, keep = waits[:-limit], waits[-limit:]
                    inst.sync_info = si.__replace__(on_wait=keep)
                    for w in extra:
                        nop = InstNoOp(
                            name=f"{inst.name}-wsplit{nid[0]}", ins=[], outs=[])
                        nid[0] += 1
                        nop.engine = inst.engine
                        nop.sync_info = mybir.SyncInfo(
                            on_wait=[w], on_update=[])
                        out.append(nop)
                    changed = True
                out.append(inst)
            if changed:
                blk.instructions = out


def _patch_ldw_opt():
    """walrus ships with --enable-ldw-opt=false; enabling it lets the PE
    pipeline LDWEIGHTS with in-flight matmuls (background weight buffer),
    hiding the ~100ns weight-load per matmul."""
    from concourse import bass_utils
    if getattr(bass_utils.run_command, "_ldwopt", False):
        return
    orig = bass_utils.run_command

    def run_command_ldwopt(cmd, *a, **kw):
        cmd = [c.replace("--enable-ldw-opt=false", "--enable-ldw-opt=true")
               if isinstance(c, str) else c for c in cmd]
        return orig(cmd, *a, **kw)

    run_command_ldwopt._ldwopt = True
    bass_utils.run_command = run_command_ldwopt


def build_nc():
    """Build the per-core Bass/Tile program."""
    _ensure_paths()
    if os.environ.get("ATTN_LDW_OPT", "0") == "1":
        _patch_ldw_opt()
    import concourse.bass as bass
    import concourse.mybir as mybir
    import concourse.tile as tile
    from contextlib import ExitStack

    f32 = mybir.dt.float32
    bf16 = mybir.dt.bfloat16

    nc = bass.Bass("TRN2", target_bir_lowering=False, debug=False,
                   num_devices=NCORES)

    xT_ext = nc.declare_dram_parameter("xT", [C, N], f32, isOutput=False)
    qkwT_ext = nc.declare_dram_parameter("qkwT", [C, 2 * C], f32, isOutput=False)
    qkb_ext = nc.declare_dram_parameter("qkb", [P, 2 * C // P], f32, isOutput=False)
    vwT_ext = nc.declare_dram_parameter("vwT", [C, C], f32, isOutput=False)
    vb_ext = nc.declare_dram_parameter("vb", [1, C], f32, isOutput=False)
    at_ext = nc.declare_dram_parameter(
        "at", [NPAIR, NQT, NKT, P, 2 * QW], f32, isOutput=False)
    pwT_ext = nc.declare_dram_parameter("pwT", [C, C], f32, isOutput=False)
    pb_ext = nc.declare_dram_parameter("pb", [P, C // P], f32, isOutput=False)
    out_ext = nc.declare_dram_parameter("out", [C, N], f32, isOutput=True)

    NQK = 2 * C // P   # 12 cout tiles for q|k

    with tile.TileContext(nc, num_cores=NCORES) as tc, ExitStack() as ctx:
        consts = ctx.enter_context(tc.tile_pool(name="consts", bufs=1))
        persist = ctx.enter_context(tc.tile_pool(name="persist", bufs=1))
        attn_pool = ctx.enter_context(tc.tile_pool(name="attnout", bufs=1))
        epool = ctx.enter_context(tc.tile_pool(name="epool", bufs=2))
        at1 = ctx.enter_context(tc.tile_pool(name="at1", bufs=6))
        atbf1 = ctx.enter_context(tc.tile_pool(name="atbf1", bufs=4))
        small = ctx.enter_context(tc.tile_pool(name="small", bufs=3))

        qkb_sb = consts.tile([P, NQK], f32)
        nc.sync.dma_start(qkb_sb[:], qkb_ext[:])
        pb_sb = consts.tile([P, NCIN], f32)
        nc.sync.dma_start(pb_sb[:], pb_ext[:])
        vbf_sb = consts.tile([1, C], f32)
        nc.sync.dma_start(vbf_sb[:], vb_ext[:])
        vb_sb = consts.tile([1, C], bf16)
        nc.vector.tensor_copy(vb_sb[:], vbf_sb[:])
        ones_sb = consts.tile([1, P], bf16)
        nc.any.memset(ones_sb[:], 1.0)
        ones64_sb = consts.tile([1, 64], f32)
        nc.any.memset(ones64_sb[:], 1.0)

        # persistent activations (bf16 matmul operands)
        qkT_sb = persist.tile([P, NQK, N], bf16)      # [q|k]^T: cout x tokens
        vp_sb = persist.tile([P, H, NKT, 65], bf16)   # [V_h | 1] stationary
        nc.any.memset(vp_sb[:, :, :, 64:65], 1.0)
        pw_sb = persist.tile([P, NCIN, C], bf16)      # proj weights (bf16)
        attn_sb = attn_pool.tile([P, NCIN, N], bf16)  # attention out^T

        # ---------------- phase 1: qkv projections ----------------
        with tc.tile_pool(name="ph1", bufs=1) as ph1, \
             tc.tile_pool(name="stgp", bufs=2) as stgp, \
             tc.tile_pool(name="pp_qk", bufs=2, space="PSUM") as pp_qk, \
             tc.tile_pool(name="pp_v", bufs=2, space="PSUM") as pp_v:
            xT_sb = ph1.tile([P, NCIN, N], bf16)
            qkw_sb = ph1.tile([P, NCIN, 2 * C], bf16)
            vw_sb = ph1.tile([P, NCIN, C], bf16)
            # staged f32 loads (double-buffered, per-kchunk) casted into
            # bf16 tensors, so matmuls start before all weights land
            xT_r = xT_ext.rearrange("(c p) t -> p c t", p=P)
            qkw_r = qkwT_ext.rearrange("(c p) n -> p c n", p=P)
            vw_r = vwT_ext.rearrange("(c p) n -> p c n", p=P)
            pw_r = pwT_ext.rearrange("(c p) n -> p c n", p=P)
            loads = []
            for kc in range(NCIN):
                loads.append((xT_r[:, kc, :], xT_sb[:, kc, :], N))
                loads.append((vw_r[:, kc, :], vw_sb[:, kc, :], C))
            for kc in range(NCIN):
                loads.append((qkw_r[:, kc, :], qkw_sb[:, kc, :], 2 * C))
            for kc in range(NCIN):
                loads.append((pw_r[:, kc, :], pw_sb[:, kc, :], C))
            for li, (src_ap, dst_ap, w) in enumerate(loads):
                stg = stgp.tile([P, 2 * C], f32, tag="stage")
                nc.sync.dma_start(stg[:, 0:w], src_ap)
                if li < 2 * NCIN:   # xT/vw gate the V matmuls: use idle ACT
                    nc.scalar.copy(dst_ap, stg[:, 0:w])
                else:
                    nc.vector.tensor_copy(dst_ap, stg[:, 0:w])

            # V[tt] = xT_slice.T @ vwT (+ ones x vb rank-1 bias), cast bf16
            for tt in range(NKT):
                ps = pp_v.tile([P, C], f32, tag="v")
                for kc in range(NCIN):
                    for (n0, nw) in ((0, QW), (QW, C - QW)):
                        nc.tensor.matmul(
                            ps[:, n0:n0 + nw],
                            xT_sb[:, kc, tt * P:(tt + 1) * P],
                            vw_sb[:, kc, n0:n0 + nw],
                            start=(kc == 0), stop=False,
                            skip_group_check=True)
                for (n0, nw) in ((0, QW), (QW, C - QW)):
                    nc.tensor.matmul(
                        ps[:, n0:n0 + nw],
                        ones_sb[0:1, 0:P],
                        vb_sb[0:1, n0:n0 + nw],
                        start=False, stop=True,
                        skip_group_check=True)
                nc.scalar.copy(
                    vp_sb[:, :, tt, 0:64],
                    ps.rearrange("p (h d) -> p h d", d=64))

            # qkT[ct] = qkwT_slice.T @ xT  (+ per-partition bias, cast bf16)
            for ct in range(NQK):
                ps = pp_qk.tile([P, N], f32, tag="qk")
                for kc in range(NCIN):
                    for qh in range(NQT):
                        nc.tensor.matmul(
                            ps[:, qh * QW:(qh + 1) * QW],
                            qkw_sb[:, kc, ct * P:(ct + 1) * P],
                            xT_sb[:, kc, qh * QW:(qh + 1) * QW],
                            start=(kc == 0), stop=(kc == NCIN - 1),
                            skip_group_check=True)
                nc.scalar.activation(
                    qkT_sb[:, ct, :], ps[:, :],
                    mybir.ActivationFunctionType.Identity,
                    bias=qkb_sb[:, ct:ct + 1])

        # ---------------- phase 2: attention ----------------
        with tc.tile_pool(name="at2", bufs=10) as at2, \
             tc.tile_pool(name="atbf2", bufs=7) as atbf2, \
             tc.tile_pool(name="pp_st", bufs=2, space="PSUM") as pp_st, \
             tc.tile_pool(name="pp_ev", bufs=2, space="PSUM") as pp_ev, \
             tc.tile_pool(name="pp_av", bufs=1, space="PSUM") as pp_av, \
             tc.tile_pool(name="pp_r", bufs=1, space="PSUM") as pp_r:
            for pr in range(NPAIR):
                h1, h2 = 2 * pr, 2 * pr + 1
                for qt in range(NQT):
                    q0 = qt * QW
                    # --- scores + exp: E^T strips [k, q] for both heads ---
                    e_sb = epool.tile([P, NKT, 2 * QW], bf16, tag="e")
                    for kt in range(NKT):
                        st = pp_st.tile([P, 2 * QW], f32, tag="st")
                        k0 = kt * P
                        nc.tensor.matmul(
                            st[:, 0:QW],
                            qkT_sb[0:64, NPAIR + pr, k0:k0 + P],
                            qkT_sb[0:64, pr, q0:q0 + QW],
                            start=True, stop=True)
                        nc.tensor.matmul(
                            st[:, QW:2 * QW],
                            qkT_sb[64:128, NPAIR + pr, k0:k0 + P],
                            qkT_sb[64:128, pr, q0:q0 + QW],
                            start=True, stop=True)
                        nc.scalar.activation(
                            e_sb[:, kt, :], st[:, :],
                            mybir.ActivationFunctionType.Exp, scale=SCALE)

                    # --- E@v (+rowsum via ones col) and A@v ---
                    psE1 = pp_ev.tile([65, QW], f32, tag="ev")
                    psE2 = pp_ev.tile([65, QW], f32, tag="ev")
                    psA = pp_av.tile([P, QW], f32, tag="av")
                    for kt in range(NKT):
                        ap_f = at1 if pr == 0 else at2
                        ap_b = atbf1 if pr == 0 else atbf2
                        at_f = ap_f.tile([P, 2 * QW], f32, tag="at")
                        nc.sync.dma_start(at_f[:], at_ext[pr, qt, kt])
                        at = ap_b.tile([P, 2 * QW], bf16, tag="atb")
                        nc.vector.tensor_copy(at[:], at_f[:])
                        st_flags = dict(start=(kt == 0), stop=(kt == NKT - 1))
                        nc.tensor.matmul(
                            psE1[:, :], vp_sb[:, h1, kt, :],
                            e_sb[:, kt, 0:QW], **st_flags)
                        nc.tensor.matmul(
                            psE2[:, :], vp_sb[:, h2, kt, :],
                            e_sb[:, kt, QW:2 * QW], **st_flags)
                        nc.tensor.matmul(
                            psA[0:64, :], vp_sb[:, h1, kt, 0:64],
                            at[:, 0:QW], **st_flags)
                        nc.tensor.matmul(
                            psA[64:128, :], vp_sb[:, h2, kt, 0:64],
                            at[:, QW:2 * QW], **st_flags)

                    # --- epilogue: out_h = E@v * (1/rowsum) + A@v ---
                    for hi, psE in ((0, psE1), (1, psE2)):
                        pa, pz = hi * 64, hi * 64 + 64
                        lns_sb = small.tile([1, QW], f32, tag="lns")
                        nc.scalar.activation(
                            lns_sb[:], psE[64:65, :],
                            mybir.ActivationFunctionType.Ln)
                        r_sb = small.tile([1, QW], f32, tag="r")
                        nc.scalar.activation(
                            r_sb[:], lns_sb[:],
                            mybir.ActivationFunctionType.Exp, scale=-1.0)
                        psR = pp_r.tile([64, QW], f32, tag="rp")
                        nc.tensor.matmul(psR[:, :], ones64_sb[:, :], r_sb[:, :],
                                         start=True, stop=True)
                        rb_sb = small.tile([64, QW], f32, tag="rb")
                        nc.vector.tensor_copy(rb_sb[:], psR[:, :])
                        dst = attn_sb[pa:pz, pr, q0:q0 + QW]
                        nc.vector.tensor_mul(dst, psE[0:64, :], rb_sb[:])
                        nc.vector.tensor_add(dst, dst, psA[pa:pz, :])

        # ---------------- phase 3: output projection ----------------
        with tc.tile_pool(name="ph3o", bufs=2) as ph3o, \
             tc.tile_pool(name="pp_p", bufs=2, space="PSUM") as pp_p:
            out_r = out_ext.rearrange("(c p) t -> p c t", p=P)
            for ct in range(NCIN):
                ps = pp_p.tile([P, N], f32, tag="pp")
                for kc in range(NCIN):
                    for qh in range(NQT):
                        nc.tensor.matmul(
                            ps[:, qh * QW:(qh + 1) * QW],
                            pw_sb[:, kc, ct * P:(ct + 1) * P],
                            attn_sb[:, kc, qh * QW:(qh + 1) * QW],
                            start=(kc == 0), stop=(kc == NCIN - 1),
                            skip_group_check=True)
                o_sb = ph3o.tile([P, N], f32, tag="o")
                nc.vector.tensor_scalar_add(o_sb[:], ps[:], pb_sb[:, ct:ct + 1])
                nc.sync.dma_start(out_r[:, ct, :], o_sb[:])

    if os.environ.get("ATTN_FUSE_LDW", "0") == "1":
        _fuse_ldweights(nc)
    if os.environ.get("ATTN_DEDUP_LDW", "1") == "1":
        _dedup_ldweights(nc)
    _split_excess_waits(nc)
    return nc


def make_in_maps(x, qkv_w, qkv_b, static_a, proj_w, proj_b):
    """Host-side sharding / layout prep. One batch element per core."""
    x = np.asarray(x, dtype=np.float32)
    qkv_w = np.asarray(qkv_w, dtype=np.float32)
    qkv_b = np.asarray(qkv_b, dtype=np.float32)
    static_a = np.asarray(static_a, dtype=np.float32)
    proj_w = np.asarray(proj_w, dtype=np.float32)
    proj_b = np.asarray(proj_b, dtype=np.float32)

    qkwT = np.ascontiguousarray(qkv_w[0:2 * C].T)            # [768, 1536]
    qkb = np.ascontiguousarray(qkv_b[0:2 * C].reshape(2 * C // P, P).T)
    vwT = np.ascontiguousarray(qkv_w[2 * C:3 * C].T)         # [768, 768]
    vb = np.ascontiguousarray(qkv_b[2 * C:3 * C].reshape(1, C))
    # A^T strips, contiguous per (pair, qtile, ktile): [6, 2, 8, 128, 1024]
    # at[pr, qt, kt, :, 0:512] = A^T[2pr][kt tile, qt tile], [..., 512:] = head 2pr+1
    atT = static_a[0].transpose(0, 2, 1)                      # [H, k, q]
    at = np.ascontiguousarray(
        atT.reshape(NPAIR, 2, NKT, P, NQT, QW).transpose(0, 4, 2, 3, 1, 5)
        .reshape(NPAIR, NQT, NKT, P, 2 * QW))
    pwT = np.ascontiguousarray(proj_w.T)
    pb = np.ascontiguousarray(proj_b.reshape(C // P, P).T)

    shared = {"qkwT": qkwT, "qkb": qkb, "vwT": vwT, "vb": vb,
              "at": at, "pwT": pwT, "pb": pb}
    in_maps = []
    for b in range(B):
        m = dict(shared)
        m["xT"] = np.ascontiguousarray(x[b].T)
        in_maps.append(m)
    return in_maps


_NC_CACHE = {}


def _get_nc():
    if "nc" not in _NC_CACHE:
        _NC_CACHE["nc"] = build_nc()
    return _NC_CACHE["nc"]


def kernel(x, qkv_w, qkv_b, static_a, proj_w, proj_b):
    _ensure_paths()
    from concourse.bass_utils import run_bass_kernel_spmd

    nc = _get_nc()
    in_maps = make_in_maps(x, qkv_w, qkv_b, static_a, proj_w, proj_b)
    res = run_bass_kernel_spmd(nc, in_maps, core_ids=list(range(NCORES)))
    out = np.empty((B, N, C), dtype=np.float32)
    for b in range(B):
        out[b] = res.results[b]["out"].T
    return out


# revision 31
# speedup vs baseline: 1.1903x; 1.1502x over previous
"""Trainium2 Bass kernel for nn_Attention_72438918414857.

Reference computation (B=8, N=1024, C=768, H=12, D=64):
    qkv = (x @ qkv_w.T + qkv_b) -> q, k, v per head
    attn = softmax(q @ k.T / sqrt(D)) + static_a   (bias added AFTER softmax)
    out = (attn @ v) merged-heads @ proj_w.T + proj_b

Sharding: data-parallel over batch -- one batch element per NeuronCore,
weights + static_a replicated. No collectives needed.

Math used on-chip (per batch, per head), everything transposed so each
matmul gets its contraction dim on partitions with no on-chip transposes:
    qkT = [Wq;Wk]^T-proj of x  ->  [cout, t] layout
    E^T = exp(K_h^T.T @ Q_h^T * D^-0.5)           [k, q] strips
    out_h^T = ([V_h|1].T @ E^T) -> rows 0..63 = E@v, row 64 = rowsum(E)
    attn_h^T = (E@v) * (1/rowsum) + V_h.T @ A_h^T
where static_a is pre-transposed on host to A^T[h, k, q].  The softmax
normalization is applied to the [64, q] output instead of the [k, q]
matrix; no max-subtraction is needed (|scores*scale| < ~3).

Matmuls run in bf16 (fp32 PE matmul is 4x slower); PSUM accumulation is
fp32.  bf16 rounding of operands keeps rel-err ~1e-3, well under the
2e-2 gate.
"""

import os
import sys

import numpy as np

B, N, C = 8, 1024, 768
H, D = 12, 64
NCORES = 8
P = 128
QW = 512          # q tile width (PSUM bank = 512 f32)
NQT = N // QW     # 2 q tiles
NKT = N // P      # 8 k tiles
NCIN = C // P     # 6 c_in chunks
NPAIR = H // 2    # 6 head pairs
SCALE = float(D) ** -0.5

_REPO = "/opt/trn_rl_repo"


def _ensure_paths():
    if _REPO not in sys.path:
        sys.path.insert(0, _REPO)


def _fuse_ldweights(nc):
    """Tile splits each matmul into Ldweights + Matmult (moving the input
    waits onto the Ldweights).  The Matmult still carries the weights
    operand, so the standalone Ldweights is droppable: delete it and move
    its waits/updates onto the matmul.  This makes every matmul
    self-loading, which walrus's LDW optimization (background weight
    buffer pipelining) requires."""
    import concourse.mybir as mybir

    for fn in nc.m.functions:
        for blk in fn.blocks:
            out = []
            pend_w, pend_u = [], []
            changed = False
            for inst in blk.instructions:
                op = str(inst.opcode)
                if op == "Ldweights":
                    si = inst.sync_info
                    if si:
                        pend_w.extend(si.on_wait or [])
                        pend_u.extend(si.on_update or [])
                    changed = True
                    continue
                if op == "Matmult" and (pend_w or pend_u):
                    si = inst.sync_info
                    ow = list(si.on_wait or []) if si else []
                    ou = list(si.on_update or []) if si else []
                    inst.sync_info = mybir.SyncInfo(
                        on_wait=pend_w + ow, on_update=pend_u + ou)
                    pend_w, pend_u = [], []
                out.append(inst)
            assert not pend_w and not pend_u, "dangling ldweights sync"
            if changed:
                blk.instructions = out


def _dedup_ldweights(nc):
    """Delete an Ldweights whose weights AP + tile geometry match the
    immediately preceding Ldweights on the PE stream (the weights are
    still resident in the array); its waits/updates move to the next
    instruction."""
    import concourse.mybir as mybir

    def sig(inst):
        ap = inst.ins[0]
        return (str(ap), str(getattr(inst, "tile_position", None)),
                str(getattr(inst, "tile_size", None)))

    for fn in nc.m.functions:
        for blk in fn.blocks:
            out = []
            last_sig = None
            pend_w, pend_u = [], []
            changed = False
            for inst in blk.instructions:
                op = str(inst.opcode)
                if op == "Ldweights":
                    s_ = sig(inst)
                    if s_ == last_sig:
                        si = inst.sync_info
                        if si:
                            pend_w.extend(si.on_wait or [])
                            pend_u.extend(si.on_update or [])
                        changed = True
                        continue
                    last_sig = s_
                elif op == "Matmult":
                    pass          # matmuls don't disturb loaded weights
                elif op in ("NoOp", "EventSemaphore"):
                    pass
                else:
                    last_sig = None
                if pend_w or pend_u:
                    si = inst.sync_info
                    ow = list(si.on_wait or []) if si else []
                    ou = list(si.on_update or []) if si else []
                    inst.sync_info = mybir.SyncInfo(
                        on_wait=pend_w + ow, on_update=pend_u + ou)
                    pend_w, pend_u = [], []
                out.append(inst)
            assert not pend_w and not pend_u
            if changed:
                blk.instructions = out


def _split_excess_waits(nc):
    """The TRN2 walrus codegen allows only 1 sem-wait command per
    instruction.  Tile's sem-assigner can emit more (one per logical
    proc a tile depends on).
    Move the excess onto freshly inserted same-engine NoOps placed just
    before the instruction -- engines execute in order, so waiting on a
    preceding NoOp is equivalent."""
    import concourse.mybir as mybir
    from bass_rust import InstNoOp

    nid = [0]
    for fn in nc.m.functions:
        for blk in fn.blocks:
            out = []
            changed = False
            for inst in blk.instructions:
                si = inst.sync_info
                waits = list(si.on_wait) if si and si.on_wait else []
                limit = 1
                if len(waits) > limit:
                    extra, keep = waits[:-limit], waits[-limit:]
                    inst.sync_info = si.__replace__(on_wait=keep)
                    for w in extra:
                        nop = InstNoOp(
                            name=f"{inst.name}-wsplit{nid[0]}", ins=[], outs=[])
                        nid[0] += 1
                        nop.engine = inst.engine
                        nop.sync_info = mybir.SyncInfo(
                            on_wait=[w], on_update=[])
                        out.append(nop)
                    changed = True
                out.append(inst)
            if changed:
                blk.instructions = out


def _patch_ldw_opt():
    """walrus ships with --enable-ldw-opt=false; enabling it lets the PE
    pipeline LDWEIGHTS with in-flight matmuls (background weight buffer),
    hiding the ~100ns weight-load per matmul."""
    from concourse import bass_utils
    if getattr(bass_utils.run_command, "_ldwopt", False):
        return
    orig = bass_utils.run_command

    def run_command_ldwopt(cmd, *a, **kw):
        cmd = [c.replace("--enable-ldw-opt=false", "--enable-ldw-opt=true")
               if isinstance(c, str) else c for c in cmd]
        return orig(cmd, *a, **kw)

    run_command_ldwopt._ldwopt = True
    bass_utils.run_command = run_command_ldwopt


def _patch_act_tables():
    """Force Bacc's activation-table chooser to the single set that
    contains every function this kernel uses (exp, ln, identity, copy),
    so only one ACT_TABLE_LOAD (~2.7us each) is emitted instead of
    ping-ponging between the exp and ln sets at every softmax epilogue."""
    import concourse.hw_specs as hw_specs
    import concourse.mybir as mybir
    if getattr(hw_specs.get_activation_tables, "_attn_patched", False):
        return
    orig = hw_specs.get_activation_tables
    keep = {mybir.ActivationFunctionType.Exp, mybir.ActivationFunctionType.Ln,
            mybir.ActivationFunctionType.Identity,
            mybir.ActivationFunctionType.Copy}

    import functools

    @functools.cache
    def patched(module_arch):
        tables = dict(orig(module_arch))
        out = {}
        for name, fns in tables.items():
            if name == "natural_log_exp_and_others":
                out[name] = fns
            else:
                out[name] = fns - keep
        return out

    patched._attn_patched = True
    hw_specs.get_activation_tables = patched
    import concourse.bacc as bacc_mod
    bacc_mod.get_activation_tables = patched


def build_nc():
    """Build the per-core Bass/Tile program."""
    _ensure_paths()
    _patch_act_tables()
    if os.environ.get("ATTN_LDW_OPT", "0") == "1":
        _patch_ldw_opt()
    import concourse.bass as bass
    import concourse.mybir as mybir
    import concourse.tile as tile
    from concourse import bacc
    from contextlib import ExitStack

    f32 = mybir.dt.float32
    bf16 = mybir.dt.bfloat16

    nc = bacc.Bacc("TRN2", target_bir_lowering=False, debug=False,
                   num_devices=NCORES)

    xT_ext = nc.declare_dram_parameter("xT", [C, N], f32, isOutput=False)
    qkwT_ext = nc.declare_dram_parameter("qkwT", [C, 2 * C], f32, isOutput=False)
    qkb_ext = nc.declare_dram_parameter("qkb", [P, 2 * C // P], f32, isOutput=False)
    vwT_ext = nc.declare_dram_parameter("vwT", [C, C], f32, isOutput=False)
    vb_ext = nc.declare_dram_parameter("vb", [1, C], f32, isOutput=False)
    at_ext = nc.declare_dram_parameter(
        "at", [NPAIR, NQT, NKT, P, 2 * QW], f32, isOutput=False)
    pwT_ext = nc.declare_dram_parameter("pwT", [C, C], f32, isOutput=False)
    pb_ext = nc.declare_dram_parameter("pb", [P, C // P], f32, isOutput=False)
    out_ext = nc.declare_dram_parameter("out", [C, N], f32, isOutput=True)

    NQK = 2 * C // P   # 12 cout tiles for q|k

    with tile.TileContext(nc, num_cores=NCORES) as tc, ExitStack() as ctx:
        consts = ctx.enter_context(tc.tile_pool(name="consts", bufs=1))
        persist = ctx.enter_context(tc.tile_pool(name="persist", bufs=1))
        attn_pool = ctx.enter_context(tc.tile_pool(name="attnout", bufs=1))
        epool = ctx.enter_context(tc.tile_pool(name="epool", bufs=2))
        atp = ctx.enter_context(tc.tile_pool(name="atp", bufs=6))
        atbf = ctx.enter_context(tc.tile_pool(name="atbf", bufs=5))
        small = ctx.enter_context(tc.tile_pool(name="small", bufs=2))

        qkb_sb = consts.tile([P, NQK], f32)
        nc.sync.dma_start(qkb_sb[:], qkb_ext[:])
        pb_sb = consts.tile([P, NCIN], f32)
        nc.sync.dma_start(pb_sb[:], pb_ext[:])
        vbf_sb = consts.tile([1, C], f32)
        nc.sync.dma_start(vbf_sb[:], vb_ext[:])
        vb_sb = consts.tile([1, C], bf16)
        nc.vector.tensor_copy(vb_sb[:], vbf_sb[:])
        ones_sb = consts.tile([1, P], bf16)
        nc.any.memset(ones_sb[:], 1.0)
        ones64_sb = consts.tile([1, 64], f32)
        nc.any.memset(ones64_sb[:], 1.0)

        # persistent activations (bf16 matmul operands)
        # per-pair q/k tensors [P, 2 (q|k), N], written right before the
        # pair's attention work so qkT matmuls interleave with attention
        qkT_prs = [persist.tile([P, 2, N], bf16, tag=f"qkt{p}",
                                name=f"qkt{p}")
                   for p in range(NPAIR)]
        vp_sb = persist.tile([P, H, NKT, 65], bf16)   # [V_h | 1] stationary
        nc.any.memset(vp_sb[:, :, :, 64:65], 1.0)
        pw_sb = persist.tile([P, NCIN, C], bf16)      # proj weights
        attn_sb = attn_pool.tile([P, NCIN, N], bf16)  # attention out^T

        with tc.tile_pool(name="ph1", bufs=1) as ph1, \
             tc.tile_pool(name="stgp", bufs=6) as stgp:
            xT_sb = ph1.tile([P, NCIN, N], bf16)
            qkw_sb = ph1.tile([P, NCIN, 2 * C], bf16)
            vw_sb = ph1.tile([P, NCIN, C], bf16)
            # staged f32 loads (double-buffered, per-kchunk) casted into
            # bf16 tensors, so matmuls start before all weights land
            xT_r = xT_ext.rearrange("(c p) t -> p c t", p=P)
            qkw_r = qkwT_ext.rearrange("(c p) n -> p c n", p=P)
            vw_r = vwT_ext.rearrange("(c p) n -> p c n", p=P)
            pw_r = pwT_ext.rearrange("(c p) n -> p c n", p=P)
            loads = []
            for kc in range(NCIN):
                loads.append((xT_r[:, kc, :], xT_sb[:, kc, :], N))
                loads.append((vw_r[:, kc, :], vw_sb[:, kc, :], C))
            for kc in range(NCIN):
                loads.append((qkw_r[:, kc, :], qkw_sb[:, kc, :], 2 * C))
            for kc in range(NCIN):
                loads.append((pw_r[:, kc, :], pw_sb[:, kc, :], C))
            for src_ap, dst_ap, w in loads:
                for w0 in range(0, w, QW):
                    pw_ = min(QW, w - w0)
                    stg = stgp.tile([P, QW], f32, tag="stage")
                    nc.sync.dma_start(stg[:, 0:pw_], src_ap[:, w0:w0 + pw_])
                    nc.vector.tensor_copy(
                        dst_ap[:, w0:w0 + pw_], stg[:, 0:pw_])

            # ---- V (kc-outer so matmuls start with the first chunks) ----
            with tc.tile_pool(name="pp_v", bufs=2, space="PSUM") as pp_v:
                for grp in range(4):
                    tts = (2 * grp, 2 * grp + 1)
                    pss = {}
                    for tt in tts:
                        pss[tt] = pp_v.tile([P, C], f32, tag="v",
                                            name=f"vps{tt}")
                    for kc in range(NCIN):
                        for tt in tts:
                            for (n0, nw) in ((0, QW), (QW, C - QW)):
                                nc.tensor.matmul(
                                    pss[tt][:, n0:n0 + nw],
                                    xT_sb[:, kc, tt * P:(tt + 1) * P],
                                    vw_sb[:, kc, n0:n0 + nw],
                                    start=(kc == 0), stop=False,
                                    skip_group_check=True)
                    for tt in tts:
                        for (n0, nw) in ((0, QW), (QW, C - QW)):
                            nc.tensor.matmul(
                                pss[tt][:, n0:n0 + nw],
                                ones_sb[0:1, 0:P],
                                vb_sb[0:1, n0:n0 + nw],
                                start=False, stop=True,
                                skip_group_check=True)
                        nc.scalar.copy(
                            vp_sb[:, :, tt, 0:64],
                            pss[tt].rearrange("p (h d) -> p h d", d=64))

            # ---- attention (+ interleaved qkT groups) ----
            with tc.tile_pool(name="pp_st", bufs=2, space="PSUM") as pp_st, \
                 tc.tile_pool(name="pp_ev", bufs=2, space="PSUM") as pp_ev, \
                     tc.tile_pool(name="pp_av", bufs=2, space="PSUM") as pp_av:

                def qkt_group(pr):
                    for qki, ct in ((0, pr), (1, NPAIR + pr)):
                        ps = pp_st.tile([P, N], f32, tag="st",
                                        name=f"qk{ct}")
                        for kc in range(NCIN):
                            for qh in range(NQT):
                                nc.tensor.matmul(
                                    ps[:, qh * QW:(qh + 1) * QW],
                                    qkw_sb[:, kc, ct * P:(ct + 1) * P],
                                    xT_sb[:, kc, qh * QW:(qh + 1) * QW],
                                    start=(kc == 0), stop=(kc == NCIN - 1),
                                    skip_group_check=True)
                        nc.scalar.activation(
                            qkT_prs[pr][:, qki, :], ps[:, :],
                            mybir.ActivationFunctionType.Identity,
                            bias=qkb_sb[:, ct:ct + 1])

                def attention_pair(pr):
                    h1, h2 = 2 * pr, 2 * pr + 1
                    for qt in range(NQT):
                        q0 = qt * QW
                        # scores + exp: E^T strips [k, q] for both heads
                        e_sb = epool.tile([P, NKT, 2 * QW], bf16, tag="e",
                                          name=f"e{pr}_{qt}")
                        for kt in range(NKT):
                            st = pp_st.tile([P, 2 * QW], f32, tag="st",
                                            name=f"st{pr}_{qt}_{kt}")
                            k0 = kt * P
                            nc.tensor.matmul(
                                st[:, 0:QW],
                                qkT_prs[pr][0:64, 1, k0:k0 + P],
                                qkT_prs[pr][0:64, 0, q0:q0 + QW],
                                start=True, stop=True)
                            nc.tensor.matmul(
                                st[:, QW:2 * QW],
                                qkT_prs[pr][64:128, 1, k0:k0 + P],
                                qkT_prs[pr][64:128, 0, q0:q0 + QW],
                                start=True, stop=True)
                            nc.scalar.activation(
                                e_sb[:, kt, :], st[:, :],
                                mybir.ActivationFunctionType.Exp, scale=SCALE)

                        # E@v (+rowsum via ones col) and A@v
                        psE1 = pp_ev.tile([P, QW], f32, tag="ev",
                                          name=f"ev1_{pr}_{qt}")
                        psE2 = pp_ev.tile([P, QW], f32, tag="ev",
                                          name=f"ev2_{pr}_{qt}")
                        psA = pp_av.tile([P, QW], f32, tag="av",
                                         name=f"av{pr}_{qt}")
                        for kt in range(NKT):
                            at_f = atp.tile([P, 2 * QW], f32, tag="at",
                                            name=f"atf{pr}_{qt}_{kt}")
                            nc.sync.dma_start(at_f[:], at_ext[pr, qt, kt])
                            at = atbf.tile([P, 2 * QW], bf16, tag="atb",
                                           name=f"atb{pr}_{qt}_{kt}")
                            nc.vector.tensor_copy(at[:], at_f[:])
                            st_flags = dict(start=(kt == 0),
                                            stop=(kt == NKT - 1))
                            nc.tensor.matmul(
                                psE1[0:65, :], vp_sb[:, h1, kt, :],
                                e_sb[:, kt, 0:QW], **st_flags)
                            nc.tensor.matmul(
                                psE2[0:65, :], vp_sb[:, h2, kt, :],
                                e_sb[:, kt, QW:2 * QW], **st_flags)
                            nc.tensor.matmul(
                                psA[0:64, :], vp_sb[:, h1, kt, 0:64],
                                at[:, 0:QW], **st_flags)
                            nc.tensor.matmul(
                                psA[64:128, :], vp_sb[:, h2, kt, 0:64],
                                at[:, QW:2 * QW], **st_flags)

                        # epilogue: out_h = E@v * (1/rowsum) + A@v
                        for hi, psE in ((0, psE1), (1, psE2)):
                            pa, pz = hi * 64, hi * 64 + 64
                            lns_sb = small.tile([1, QW], f32, tag="lns",
                                                name=f"ln{pr}_{qt}_{hi}")
                            nc.scalar.activation(
                                lns_sb[:], psE[64:65, :],
                                mybir.ActivationFunctionType.Ln)
                            r_sb = small.tile([1, QW], f32, tag="r",
                                              name=f"r{pr}_{qt}_{hi}")
                            nc.scalar.activation(
                                r_sb[:], lns_sb[:],
                                mybir.ActivationFunctionType.Exp, scale=-1.0)
                            # broadcast r into the unused upper partitions
                            # of this head's own ev bank (s row already
                            # consumed by the Ln above)
                            nc.tensor.matmul(psE[64:128, :], ones64_sb[:, :],
                                             r_sb[:, :], start=True, stop=True)
                            rb_sb = small.tile([64, QW], f32, tag="rb",
                                               name=f"rb{pr}_{qt}_{hi}")
                            nc.vector.tensor_copy(rb_sb[:], psE[64:128, :])
                            dst = attn_sb[pa:pz, pr, q0:q0 + QW]
                            nc.vector.tensor_mul(dst, psE[0:64, :], rb_sb[:])
                            nc.vector.tensor_add(dst, dst, psA[pa:pz, :])

                qkt_group(0)
                attention_pair(0)
                for pr in range(1, NPAIR):
                    qkt_group(pr)
                for pr in range(1, NPAIR):
                    attention_pair(pr)

                # ---- output projection ----
                with tc.tile_pool(name="ph3o", bufs=2) as ph3o:
                    out_r = out_ext.rearrange("(c p) t -> p c t", p=P)
                    for ct in range(NCIN):
                        ps = pp_st.tile([P, N], f32, tag="st",
                                        name=f"proj{ct}")
                        for kc in range(NCIN):
                            for qh in range(NQT):
                                nc.tensor.matmul(
                                    ps[:, qh * QW:(qh + 1) * QW],
                                    pw_sb[:, kc, ct * P:(ct + 1) * P],
                                    attn_sb[:, kc, qh * QW:(qh + 1) * QW],
                                    start=(kc == 0), stop=(kc == NCIN - 1),
                                    skip_group_check=True)
                        o_sb = ph3o.tile([P, N], f32, tag="o",
                                         name=f"o{ct}")
                        nc.vector.tensor_scalar_add(
                            o_sb[:], ps[:], pb_sb[:, ct:ct + 1])
                        nc.sync.dma_start(out_r[:, ct, :], o_sb[:])

    if os.environ.get("ATTN_FUSE_LDW", "0") == "1":
        _fuse_ldweights(nc)
    if os.environ.get("ATTN_DEDUP_LDW", "1") == "1":
        _dedup_ldweights(nc)
    if os.environ.get("ATTN_SPLIT_WAITS", "0") == "1":
        _split_excess_waits(nc)
    if not nc.is_finalized():
        nc.finalize()   # Bacc: move_matmul_waits + generate_event_semaphores
    return nc


def make_in_maps(x, qkv_w, qkv_b, static_a, proj_w, proj_b):
    """Host-side sharding / layout prep. One batch element per core."""
    x = np.asarray(x, dtype=np.float32)
    qkv_w = np.asarray(qkv_w, dtype=np.float32)
    qkv_b = np.asarray(qkv_b, dtype=np.float32)
    static_a = np.asarray(static_a, dtype=np.float32)
    proj_w = np.asarray(proj_w, dtype=np.float32)
    proj_b = np.asarray(proj_b, dtype=np.float32)

    qkwT = np.ascontiguousarray(qkv_w[0:2 * C].T)            # [768, 1536]
    qkb = np.ascontiguousarray(qkv_b[0:2 * C].reshape(2 * C // P, P).T)
    vwT = np.ascontiguousarray(qkv_w[2 * C:3 * C].T)         # [768, 768]
    vb = np.ascontiguousarray(qkv_b[2 * C:3 * C].reshape(1, C))
    # A^T strips, contiguous per (pair, qtile, ktile): [6, 2, 8, 128, 1024]
    # at[pr, qt, kt, :, 0:512] = A^T[2pr][kt tile, qt tile], [..., 512:] = head 2pr+1
    atT = static_a[0].transpose(0, 2, 1)                      # [H, k, q]
    at = np.ascontiguousarray(
        atT.reshape(NPAIR, 2, NKT, P, NQT, QW).transpose(0, 4, 2, 3, 1, 5)
        .reshape(NPAIR, NQT, NKT, P, 2 * QW))
    pwT = np.ascontiguousarray(proj_w.T)
    pb = np.ascontiguousarray(proj_b.reshape(C // P, P).T)

    shared = {"qkwT": qkwT, "qkb": qkb, "vwT": vwT, "vb": vb,
              "at": at, "pwT": pwT, "pb": pb}
    in_maps = []
    for b in range(B):
        m = dict(shared)
        m["xT"] = np.ascontiguousarray(x[b].T)
        in_maps.append(m)
    return in_maps


_NC_CACHE = {}


def _get_nc():
    if "nc" not in _NC_CACHE:
        _NC_CACHE["nc"] = build_nc()
    return _NC_CACHE["nc"]


def kernel(x, qkv_w, qkv_b, static_a, proj_w, proj_b):
    _ensure_paths()
    from concourse.bass_utils import run_bass_kernel_spmd

    nc = _get_nc()
    in_maps = make_in_maps(x, qkv_w, qkv_b, static_a, proj_w, proj_b)
    res = run_bass_kernel_spmd(nc, in_maps, core_ids=list(range(NCORES)))
    out = np.empty((B, N, C), dtype=np.float32)
    for b in range(B):
        out[b] = res.results[b]["out"].T
    return out


# revision 38
# speedup vs baseline: 1.2151x; 1.0209x over previous
"""Trainium2 Bass kernel for nn_Attention_72438918414857.

Reference computation (B=8, N=1024, C=768, H=12, D=64):
    qkv = (x @ qkv_w.T + qkv_b) -> q, k, v per head
    attn = softmax(q @ k.T / sqrt(D)) + static_a   (bias added AFTER softmax)
    out = (attn @ v) merged-heads @ proj_w.T + proj_b

Sharding: data-parallel over batch -- one batch element per NeuronCore,
weights + static_a replicated. No collectives needed.

Math used on-chip (per batch, per head), everything transposed so each
matmul gets its contraction dim on partitions with no on-chip transposes:
    qkT = [Wq;Wk]^T-proj of x  ->  [cout, t] layout
    E^T = exp(K_h^T.T @ Q_h^T * D^-0.5)           [k, q] strips
    out_h^T = ([V_h|1].T @ E^T) -> rows 0..63 = E@v, row 64 = rowsum(E)
    attn_h^T = (E@v) * (1/rowsum) + V_h.T @ A_h^T
where static_a is pre-transposed on host to A^T[h, k, q].  The softmax
normalization is applied to the [64, q] output instead of the [k, q]
matrix; no max-subtraction is needed (|scores*scale| < ~3).

Matmuls run in bf16 (fp32 PE matmul is 4x slower); PSUM accumulation is
fp32.  bf16 rounding of operands keeps rel-err ~1e-3, well under the
2e-2 gate.
"""

import os
import sys

import numpy as np

B, N, C = 8, 1024, 768
H, D = 12, 64
NCORES = 8
P = 128
QW = 512          # q tile width (PSUM bank = 512 f32)
NQT = N // QW     # 2 q tiles
NKT = N // P      # 8 k tiles
NCIN = C // P     # 6 c_in chunks
NPAIR = H // 2    # 6 head pairs
SCALE = float(D) ** -0.5

_REPO = "/opt/trn_rl_repo"


def _ensure_paths():
    if _REPO not in sys.path:
        sys.path.insert(0, _REPO)


def _fuse_ldweights(nc):
    """Tile splits each matmul into Ldweights + Matmult (moving the input
    waits onto the Ldweights).  The Matmult still carries the weights
    operand, so the standalone Ldweights is droppable: delete it and move
    its waits/updates onto the matmul.  This makes every matmul
    self-loading, which walrus's LDW optimization (background weight
    buffer pipelining) requires."""
    import concourse.mybir as mybir

    for fn in nc.m.functions:
        for blk in fn.blocks:
            out = []
            pend_w, pend_u = [], []
            changed = False
            for inst in blk.instructions:
                op = str(inst.opcode)
                if op == "Ldweights":
                    si = inst.sync_info
                    if si:
                        pend_w.extend(si.on_wait or [])
                        pend_u.extend(si.on_update or [])
                    changed = True
                    continue
                if op == "Matmult" and (pend_w or pend_u):
                    si = inst.sync_info
                    ow = list(si.on_wait or []) if si else []
                    ou = list(si.on_update or []) if si else []
                    inst.sync_info = mybir.SyncInfo(
                        on_wait=pend_w + ow, on_update=pend_u + ou)
                    pend_w, pend_u = [], []
                out.append(inst)
            assert not pend_w and not pend_u, "dangling ldweights sync"
            if changed:
                blk.instructions = out


def _dedup_ldweights(nc):
    """Delete an Ldweights whose weights AP + tile geometry match the
    immediately preceding Ldweights on the PE stream (the weights are
    still resident in the array); its waits/updates move to the next
    instruction."""
    import concourse.mybir as mybir

    def sig(inst):
        ap = inst.ins[0]
        return (str(ap), str(getattr(inst, "tile_position", None)),
                str(getattr(inst, "tile_size", None)))

    for fn in nc.m.functions:
        for blk in fn.blocks:
            out = []
            last_sig = None
            pend_w, pend_u = [], []
            changed = False
            for inst in blk.instructions:
                op = str(inst.opcode)
                if op == "Ldweights":
                    s_ = sig(inst)
                    if s_ == last_sig:
                        si = inst.sync_info
                        if si:
                            pend_w.extend(si.on_wait or [])
                            pend_u.extend(si.on_update or [])
                        changed = True
                        continue
                    last_sig = s_
                elif op == "Matmult":
                    pass          # matmuls don't disturb loaded weights
                elif op in ("NoOp", "EventSemaphore"):
                    pass
                else:
                    last_sig = None
                if pend_w or pend_u:
                    si = inst.sync_info
                    ow = list(si.on_wait or []) if si else []
                    ou = list(si.on_update or []) if si else []
                    inst.sync_info = mybir.SyncInfo(
                        on_wait=pend_w + ow, on_update=pend_u + ou)
                    pend_w, pend_u = [], []
                out.append(inst)
            assert not pend_w and not pend_u
            if changed:
                blk.instructions = out


def _split_excess_waits(nc):
    """The TRN2 walrus codegen allows only 1 sem-wait command per
    instruction.  Tile's sem-assigner can emit more (one per logical
    proc a tile depends on).
    Move the excess onto freshly inserted same-engine NoOps placed just
    before the instruction -- engines execute in order, so waiting on a
    preceding NoOp is equivalent."""
    import concourse.mybir as mybir
    from bass_rust import InstNoOp

    nid = [0]
    for fn in nc.m.functions:
        for blk in fn.blocks:
            out = []
            changed = False
            for inst in blk.instructions:
                si = inst.sync_info
                waits = list(si.on_wait) if si and si.on_wait else []
                limit = 1
                if len(waits) > limit:
                    extra, keep = waits[:-limit], waits[-limit:]
                    inst.sync_info = si.__replace__(on_wait=keep)
                    for w in extra:
                        nop = InstNoOp(
                            name=f"{inst.name}-wsplit{nid[0]}", ins=[], outs=[])
                        nid[0] += 1
                        nop.engine = inst.engine
                        nop.sync_info = mybir.SyncInfo(
                            on_wait=[w], on_update=[])
                        out.append(nop)
                    changed = True
                out.append(inst)
            if changed:
                blk.instructions = out


def _patch_ldw_opt():
    """walrus ships with --enable-ldw-opt=false; enabling it lets the PE
    pipeline LDWEIGHTS with in-flight matmuls (background weight buffer),
    hiding the ~100ns weight-load per matmul."""
    from concourse import bass_utils
    if getattr(bass_utils.run_command, "_ldwopt", False):
        return
    orig = bass_utils.run_command

    def run_command_ldwopt(cmd, *a, **kw):
        cmd = [c.replace("--enable-ldw-opt=false", "--enable-ldw-opt=true")
               if isinstance(c, str) else c for c in cmd]
        return orig(cmd, *a, **kw)

    run_command_ldwopt._ldwopt = True
    bass_utils.run_command = run_command_ldwopt


def _patch_act_tables():
    """Force Bacc's activation-table chooser to the single set that
    contains every function this kernel uses (exp, ln, identity, copy),
    so only one ACT_TABLE_LOAD (~2.7us each) is emitted instead of
    ping-ponging between the exp and ln sets at every softmax epilogue."""
    import concourse.hw_specs as hw_specs
    import concourse.mybir as mybir
    if getattr(hw_specs.get_activation_tables, "_attn_patched", False):
        return
    orig = hw_specs.get_activation_tables
    keep = {mybir.ActivationFunctionType.Exp, mybir.ActivationFunctionType.Ln,
            mybir.ActivationFunctionType.Identity,
            mybir.ActivationFunctionType.Copy}

    import functools

    @functools.cache
    def patched(module_arch):
        tables = dict(orig(module_arch))
        out = {}
        for name, fns in tables.items():
            if name == "natural_log_exp_and_others":
                out[name] = fns
            else:
                out[name] = fns - keep
        return out

    patched._attn_patched = True
    hw_specs.get_activation_tables = patched
    import concourse.bacc as bacc_mod
    bacc_mod.get_activation_tables = patched


def build_nc():
    """Build the per-core Bass/Tile program."""
    _ensure_paths()
    _patch_act_tables()
    if os.environ.get("ATTN_LDW_OPT", "0") == "1":
        _patch_ldw_opt()
    import concourse.bass as bass
    import concourse.mybir as mybir
    import concourse.tile as tile
    from concourse import bacc
    from contextlib import ExitStack

    f32 = mybir.dt.float32
    bf16 = mybir.dt.bfloat16

    nc = bacc.Bacc("TRN2", target_bir_lowering=False, debug=False,
                   num_devices=NCORES)

    xT_ext = nc.declare_dram_parameter("xT", [C, N], f32, isOutput=False)
    qkwT_ext = nc.declare_dram_parameter("qkwT", [C, 2 * C], f32, isOutput=False)
    qkb_ext = nc.declare_dram_parameter("qkb", [P, 2 * C // P], f32, isOutput=False)
    vwT_ext = nc.declare_dram_parameter("vwT", [C, C], f32, isOutput=False)
    vb_ext = nc.declare_dram_parameter("vb", [1, C], f32, isOutput=False)
    at_ext = nc.declare_dram_parameter(
        "at", [NPAIR, NQT, NKT, P, 2 * QW], f32, isOutput=False)
    pwT_ext = nc.declare_dram_parameter("pwT", [C, C], f32, isOutput=False)
    pb_ext = nc.declare_dram_parameter("pb", [P, C // P], f32, isOutput=False)
    out_ext = nc.declare_dram_parameter("out", [C, N], f32, isOutput=True)

    NQK = 2 * C // P   # 12 cout tiles for q|k

    with tile.TileContext(nc, num_cores=NCORES) as tc, ExitStack() as ctx:
        consts = ctx.enter_context(tc.tile_pool(name="consts", bufs=1))
        persist = ctx.enter_context(tc.tile_pool(name="persist", bufs=1))
        attn_pool = ctx.enter_context(tc.tile_pool(name="attnout", bufs=1))
        epool = ctx.enter_context(tc.tile_pool(name="epool", bufs=2))
        atp = ctx.enter_context(tc.tile_pool(name="atp", bufs=6))
        atbf = ctx.enter_context(tc.tile_pool(name="atbf", bufs=5))
        small = ctx.enter_context(tc.tile_pool(name="small", bufs=2))

        qkb_sb = consts.tile([P, NQK], f32)
        nc.sync.dma_start(qkb_sb[:], qkb_ext[:])
        pb_sb = consts.tile([P, NCIN], f32)
        nc.sync.dma_start(pb_sb[:], pb_ext[:])
        vbf_sb = consts.tile([1, C], f32)
        nc.sync.dma_start(vbf_sb[:], vb_ext[:])
        vb_sb = consts.tile([1, C], bf16)
        nc.vector.tensor_copy(vb_sb[:], vbf_sb[:])
        ones_sb = consts.tile([1, P], bf16)
        nc.any.memset(ones_sb[:], 1.0)
        ones64_sb = consts.tile([33, 64], f32)
        nc.any.memset(ones64_sb[:], 1.0)   # rows 0 and 32 used as rank-1 lhsT

        # persistent activations (bf16 matmul operands)
        # per-pair q/k tensors [P, 2 (q|k), N], written right before the
        # pair's attention work so qkT matmuls interleave with attention
        qkT_prs = [persist.tile([P, 2, N], bf16, tag=f"qkt{p}",
                                name=f"qkt{p}")
                   for p in range(NPAIR)]
        vp_sb = persist.tile([P, H, NKT, 65], bf16)   # [V_h | 1] stationary
        nc.any.memset(vp_sb[:, :, :, 64:65], 1.0)
        pw_sb = persist.tile([P, NCIN, C], bf16)      # proj weights
        attn_sb = attn_pool.tile([P, NCIN, N], bf16)  # attention out^T

        with tc.tile_pool(name="ph1", bufs=1) as ph1, \
             tc.tile_pool(name="stgp", bufs=6) as stgp:
            xT_sb = ph1.tile([P, NCIN, N], bf16)
            qkw_sb = ph1.tile([P, NCIN, 2 * C], bf16)
            vw_sb = ph1.tile([P, NCIN, C], bf16)
            # staged f32 loads (double-buffered, per-kchunk) casted into
            # bf16 tensors, so matmuls start before all weights land
            xT_r = xT_ext.rearrange("(c p) t -> p c t", p=P)
            qkw_r = qkwT_ext.rearrange("(c p) n -> p c n", p=P)
            vw_r = vwT_ext.rearrange("(c p) n -> p c n", p=P)
            pw_r = pwT_ext.rearrange("(c p) n -> p c n", p=P)
            loads = []
            for kc in range(NCIN):
                loads.append((xT_r[:, kc, :], xT_sb[:, kc, :], N))
                loads.append((vw_r[:, kc, :], vw_sb[:, kc, :], C))
            for kc in range(NCIN):
                loads.append((qkw_r[:, kc, :], qkw_sb[:, kc, :], 2 * C))
            for kc in range(NCIN):
                loads.append((pw_r[:, kc, :], pw_sb[:, kc, :], C))
            for src_ap, dst_ap, w in loads:
                for w0 in range(0, w, QW):
                    pw_ = min(QW, w - w0)
                    stg = stgp.tile([P, QW], f32, tag="stage")
                    nc.sync.dma_start(stg[:, 0:pw_], src_ap[:, w0:w0 + pw_])
                    nc.vector.tensor_copy(
                        dst_ap[:, w0:w0 + pw_], stg[:, 0:pw_])

            # ---- V (kc-outer so matmuls start with the first chunks) ----
            with tc.tile_pool(name="pp_v", bufs=2, space="PSUM") as pp_v:
                for grp in range(4):
                    tts = (2 * grp, 2 * grp + 1)
                    pss = {}
                    for tt in tts:
                        pss[tt] = pp_v.tile([P, C], f32, tag="v",
                                            name=f"vps{tt}")
                    for kc in range(NCIN):
                        for tt in tts:
                            for (n0, nw) in ((0, QW), (QW, C - QW)):
                                nc.tensor.matmul(
                                    pss[tt][:, n0:n0 + nw],
                                    xT_sb[:, kc, tt * P:(tt + 1) * P],
                                    vw_sb[:, kc, n0:n0 + nw],
                                    start=(kc == 0), stop=False,
                                    skip_group_check=True)
                    for tt in tts:
                        for (n0, nw) in ((0, QW), (QW, C - QW)):
                            nc.tensor.matmul(
                                pss[tt][:, n0:n0 + nw],
                                ones_sb[0:1, 0:P],
                                vb_sb[0:1, n0:n0 + nw],
                                start=False, stop=True,
                                skip_group_check=True)
                        nc.scalar.copy(
                            vp_sb[:, :, tt, 0:64],
                            pss[tt].rearrange("p (h d) -> p h d", d=64))

            # ---- attention (+ interleaved qkT groups) ----
            with tc.tile_pool(name="pp_st", bufs=2, space="PSUM") as pp_st, \
                 tc.tile_pool(name="pp_ev", bufs=2, space="PSUM") as pp_ev, \
                     tc.tile_pool(name="pp_av", bufs=2, space="PSUM") as pp_av:

                def qkt_group(pr):
                    for qki, ct in ((0, pr), (1, NPAIR + pr)):
                        ps = pp_st.tile([P, N], f32, tag="st",
                                        name=f"qk{ct}")
                        for kc in range(NCIN):
                            for qh in range(NQT):
                                nc.tensor.matmul(
                                    ps[:, qh * QW:(qh + 1) * QW],
                                    qkw_sb[:, kc, ct * P:(ct + 1) * P],
                                    xT_sb[:, kc, qh * QW:(qh + 1) * QW],
                                    start=(kc == 0), stop=(kc == NCIN - 1),
                                    skip_group_check=True)
                        nc.scalar.activation(
                            qkT_prs[pr][:, qki, :], ps[:, :],
                            mybir.ActivationFunctionType.Identity,
                            bias=qkb_sb[:, ct:ct + 1])

                def emit_st_step(pr, qt, e_sb, kt):
                    q0 = qt * QW
                    st = pp_st.tile([P, 2 * QW], f32, tag="st",
                                    name=f"st{pr}_{qt}_{kt}")
                    k0 = kt * P
                    nc.tensor.matmul(
                        st[:, 0:QW],
                        qkT_prs[pr][0:64, 1, k0:k0 + P],
                        qkT_prs[pr][0:64, 0, q0:q0 + QW],
                        start=True, stop=True)
                    nc.tensor.matmul(
                        st[:, QW:2 * QW],
                        qkT_prs[pr][64:128, 1, k0:k0 + P],
                        qkT_prs[pr][64:128, 0, q0:q0 + QW],
                        start=True, stop=True)
                    nc.scalar.activation(
                        e_sb[:, kt, :], st[:, :],
                        mybir.ActivationFunctionType.Exp, scale=SCALE)

                def emit_out_step(item, kt):
                    pr, qt, e_sb, psE1, psE2, psA = item
                    h1, h2 = 2 * pr, 2 * pr + 1
                    at_f = atp.tile([P, 2 * QW], f32, tag="at",
                                    name=f"atf{pr}_{qt}_{kt}")
                    nc.sync.dma_start(at_f[:], at_ext[pr, qt, kt])
                    at = atbf.tile([P, 2 * QW], bf16, tag="atb",
                                   name=f"atb{pr}_{qt}_{kt}")
                    nc.vector.tensor_copy(at[:], at_f[:])
                    st_flags = dict(start=(kt == 0), stop=(kt == NKT - 1))
                    nc.tensor.matmul(
                        psE1[0:65, :], vp_sb[:, h1, kt, :],
                        e_sb[:, kt, 0:QW], **st_flags)
                    nc.tensor.matmul(
                        psE2[0:65, :], vp_sb[:, h2, kt, :],
                        e_sb[:, kt, QW:2 * QW], **st_flags)
                    nc.tensor.matmul(
                        psA[0:64, :], vp_sb[:, h1, kt, 0:64],
                        at[:, 0:QW], **st_flags)
                    nc.tensor.matmul(
                        psA[64:128, :], vp_sb[:, h2, kt, 0:64],
                        at[:, QW:2 * QW], **st_flags)

                def emit_epilogue_act(item):
                    # reciprocal chain on ACT; runs while the next block's
                    # score matmuls keep the PE busy
                    pr, qt, e_sb, psE1, psE2, psA = item
                    rs = []
                    for hi, psE in ((0, psE1), (1, psE2)):
                        lns_sb = small.tile([1, QW], f32, tag="lns",
                                            name=f"ln{pr}_{qt}_{hi}")
                        nc.scalar.activation(
                            lns_sb[:], psE[64:65, :],
                            mybir.ActivationFunctionType.Ln)
                        r_sb = small.tile([1, QW], f32, tag="r",
                                          name=f"r{pr}_{qt}_{hi}")
                        nc.scalar.activation(
                            r_sb[:], lns_sb[:],
                            mybir.ActivationFunctionType.Exp, scale=-1.0)
                        rs.append(r_sb)
                    return rs

                def emit_epilogue_pe(item, rs):
                    pr, qt, e_sb, psE1, psE2, psA = item
                    q0 = qt * QW
                    for hi, psE in ((0, psE1), (1, psE2)):
                        pa, pz = hi * 64, hi * 64 + 64
                        nc.tensor.matmul(psE[64:128, :],
                                         ones64_sb[0:1, :],
                                         rs[hi][:, :], start=True, stop=True)
                        rb_sb = small.tile([64, QW], f32, tag="rb",
                                           name=f"rb{pr}_{qt}_{hi}")
                        nc.vector.tensor_copy(rb_sb[:], psE[64:128, :])
                        dst = attn_sb[pa:pz, pr, q0:q0 + QW]
                        nc.vector.tensor_mul(dst, psE[0:64, :], rb_sb[:])
                        nc.vector.tensor_add(dst, dst, psA[pa:pz, :])

                # software-pipelined emission: item i's ST/exp stream is
                # interleaved kt-by-kt with item i-1's E@v/A@v matmuls, so
                # the PE has dense work while ACT drains the score tiles
                items = [(pr, qt) for pr in range(NPAIR)
                         for qt in range(NQT)]
                prev = None        # item whose OUT runs in the current block
                pend = None        # (item, rs): awaiting its PE/DVE epilogue
                for pr, qt in items:
                    if qt == 0:
                        qkt_group(pr)
                    e_sb = epool.tile([P, NKT, 2 * QW], bf16, tag="e",
                                      name=f"e{pr}_{qt}")
                    # two score steps up front cover the pending epilogue's
                    # ACT reciprocal latency before its PE part is issued
                    emit_st_step(pr, qt, e_sb, 0)
                    emit_st_step(pr, qt, e_sb, 1)
                    if pend is not None:
                        emit_epilogue_pe(*pend)
                        pend = None
                    psE1 = pp_ev.tile([P, QW], f32, tag="ev",
                                      name=f"ev1_{pr}_{qt}")
                    psE2 = pp_ev.tile([P, QW], f32, tag="ev",
                                      name=f"ev2_{pr}_{qt}")
                    psA = pp_av.tile([P, QW], f32, tag="av",
                                     name=f"av{pr}_{qt}")
                    cur = (pr, qt, e_sb, psE1, psE2, psA)
                    for kt in range(NKT):
                        if kt + 2 < NKT:
                            emit_st_step(pr, qt, e_sb, kt + 2)
                        if prev is not None:
                            emit_out_step(prev, kt)
                    if prev is not None:
                        pend = (prev, emit_epilogue_act(prev))
                    prev = cur
                # drain the last item unpipelined
                for kt in range(NKT):
                    emit_out_step(prev, kt)
                if pend is not None:
                    emit_epilogue_pe(*pend)
                emit_epilogue_pe(prev, emit_epilogue_act(prev))

                # ---- output projection ----
                with tc.tile_pool(name="ph3o", bufs=2) as ph3o:
                    out_r = out_ext.rearrange("(c p) t -> p c t", p=P)
                    for ct in range(NCIN):
                        ps = pp_st.tile([P, N], f32, tag="st",
                                        name=f"proj{ct}")
                        for kc in range(NCIN):
                            for qh in range(NQT):
                                nc.tensor.matmul(
                                    ps[:, qh * QW:(qh + 1) * QW],
                                    pw_sb[:, kc, ct * P:(ct + 1) * P],
                                    attn_sb[:, kc, qh * QW:(qh + 1) * QW],
                                    start=(kc == 0), stop=(kc == NCIN - 1),
                                    skip_group_check=True)
                        o_sb = ph3o.tile([P, N], f32, tag="o",
                                         name=f"o{ct}")
                        nc.vector.tensor_scalar_add(
                            o_sb[:], ps[:], pb_sb[:, ct:ct + 1])
                        nc.sync.dma_start(out_r[:, ct, :], o_sb[:])

    if os.environ.get("ATTN_FUSE_LDW", "0") == "1":
        _fuse_ldweights(nc)
    if os.environ.get("ATTN_DEDUP_LDW", "1") == "1":
        _dedup_ldweights(nc)
    if os.environ.get("ATTN_SPLIT_WAITS", "0") == "1":
        _split_excess_waits(nc)
    if not nc.is_finalized():
        nc.finalize()   # Bacc: move_matmul_waits + generate_event_semaphores
    return nc


def make_in_maps(x, qkv_w, qkv_b, static_a, proj_w, proj_b):
    """Host-side sharding / layout prep. One batch element per core."""
    x = np.asarray(x, dtype=np.float32)
    qkv_w = np.asarray(qkv_w, dtype=np.float32)
    qkv_b = np.asarray(qkv_b, dtype=np.float32)
    static_a = np.asarray(static_a, dtype=np.float32)
    proj_w = np.asarray(proj_w, dtype=np.float32)
    proj_b = np.asarray(proj_b, dtype=np.float32)

    qkwT = np.ascontiguousarray(qkv_w[0:2 * C].T)            # [768, 1536]
    qkb = np.ascontiguousarray(qkv_b[0:2 * C].reshape(2 * C // P, P).T)
    vwT = np.ascontiguousarray(qkv_w[2 * C:3 * C].T)         # [768, 768]
    vb = np.ascontiguousarray(qkv_b[2 * C:3 * C].reshape(1, C))
    # A^T strips, contiguous per (pair, qtile, ktile): [6, 2, 8, 128, 1024]
    # at[pr, qt, kt, :, 0:512] = A^T[2pr][kt tile, qt tile], [..., 512:] = head 2pr+1
    atT = static_a[0].transpose(0, 2, 1)                      # [H, k, q]
    at = np.ascontiguousarray(
        atT.reshape(NPAIR, 2, NKT, P, NQT, QW).transpose(0, 4, 2, 3, 1, 5)
        .reshape(NPAIR, NQT, NKT, P, 2 * QW))
    pwT = np.ascontiguousarray(proj_w.T)
    pb = np.ascontiguousarray(proj_b.reshape(C // P, P).T)

    shared = {"qkwT": qkwT, "qkb": qkb, "vwT": vwT, "vb": vb,
              "at": at, "pwT": pwT, "pb": pb}
    in_maps = []
    for b in range(B):
        m = dict(shared)
        m["xT"] = np.ascontiguousarray(x[b].T)
        in_maps.append(m)
    return in_maps


_NC_CACHE = {}


def _get_nc():
    if "nc" not in _NC_CACHE:
        _NC_CACHE["nc"] = build_nc()
    return _NC_CACHE["nc"]


def kernel(x, qkv_w, qkv_b, static_a, proj_w, proj_b):
    _ensure_paths()
    from concourse.bass_utils import run_bass_kernel_spmd

    nc = _get_nc()
    in_maps = make_in_maps(x, qkv_w, qkv_b, static_a, proj_w, proj_b)
    res = run_bass_kernel_spmd(nc, in_maps, core_ids=list(range(NCORES)))
    out = np.empty((B, N, C), dtype=np.float32)
    for b in range(B):
        out[b] = res.results[b]["out"].T
    return out


# revision 41
# speedup vs baseline: 1.2221x; 1.0057x over previous
"""Trainium2 Bass kernel for nn_Attention_72438918414857.

Reference computation (B=8, N=1024, C=768, H=12, D=64):
    qkv = (x @ qkv_w.T + qkv_b) -> q, k, v per head
    attn = softmax(q @ k.T / sqrt(D)) + static_a   (bias added AFTER softmax)
    out = (attn @ v) merged-heads @ proj_w.T + proj_b

Sharding: data-parallel over batch -- one batch element per NeuronCore,
weights + static_a replicated. No collectives needed.

Math used on-chip (per batch, per head), everything transposed so each
matmul gets its contraction dim on partitions with no on-chip transposes:
    qkT = [Wq;Wk]^T-proj of x  ->  [cout, t] layout
    E^T = exp(K_h^T.T @ Q_h^T * D^-0.5)           [k, q] strips
    out_h^T = ([V_h|1].T @ E^T) -> rows 0..63 = E@v, row 64 = rowsum(E)
    attn_h^T = (E@v) * (1/rowsum) + V_h.T @ A_h^T
where static_a is pre-transposed on host to A^T[h, k, q].  The softmax
normalization is applied to the [64, q] output instead of the [k, q]
matrix; no max-subtraction is needed (|scores*scale| < ~3).

Matmuls run in bf16 (fp32 PE matmul is 4x slower); PSUM accumulation is
fp32.  bf16 rounding of operands keeps rel-err ~1e-3, well under the
2e-2 gate.
"""

import os
import sys

import numpy as np

B, N, C = 8, 1024, 768
H, D = 12, 64
NCORES = 8
P = 128
QW = 512          # q tile width (PSUM bank = 512 f32)
NQT = N // QW     # 2 q tiles
NKT = N // P      # 8 k tiles
NCIN = C // P     # 6 c_in chunks
NPAIR = H // 2    # 6 head pairs
SCALE = float(D) ** -0.5

_REPO = "/opt/trn_rl_repo"


def _ensure_paths():
    if _REPO not in sys.path:
        sys.path.insert(0, _REPO)


def _fuse_ldweights(nc):
    """Tile splits each matmul into Ldweights + Matmult (moving the input
    waits onto the Ldweights).  The Matmult still carries the weights
    operand, so the standalone Ldweights is droppable: delete it and move
    its waits/updates onto the matmul.  This makes every matmul
    self-loading, which walrus's LDW optimization (background weight
    buffer pipelining) requires."""
    import concourse.mybir as mybir

    for fn in nc.m.functions:
        for blk in fn.blocks:
            out = []
            pend_w, pend_u = [], []
            changed = False
            for inst in blk.instructions:
                op = str(inst.opcode)
                if op == "Ldweights":
                    si = inst.sync_info
                    if si:
                        pend_w.extend(si.on_wait or [])
                        pend_u.extend(si.on_update or [])
                    changed = True
                    continue
                if op == "Matmult" and (pend_w or pend_u):
                    si = inst.sync_info
                    ow = list(si.on_wait or []) if si else []
                    ou = list(si.on_update or []) if si else []
                    inst.sync_info = mybir.SyncInfo(
                        on_wait=pend_w + ow, on_update=pend_u + ou)
                    pend_w, pend_u = [], []
                out.append(inst)
            assert not pend_w and not pend_u, "dangling ldweights sync"
            if changed:
                blk.instructions = out


def _dedup_ldweights(nc):
    """Delete an Ldweights whose weights AP + tile geometry match the
    immediately preceding Ldweights on the PE stream (the weights are
    still resident in the array); its waits/updates move to the next
    instruction."""
    import concourse.mybir as mybir

    def sig(inst):
        ap = inst.ins[0]
        return (str(ap), str(getattr(inst, "tile_position", None)),
                str(getattr(inst, "tile_size", None)))

    for fn in nc.m.functions:
        for blk in fn.blocks:
            out = []
            last_sig = None
            pend_w, pend_u = [], []
            changed = False
            for inst in blk.instructions:
                op = str(inst.opcode)
                if op == "Ldweights":
                    s_ = sig(inst)
                    if s_ == last_sig:
                        si = inst.sync_info
                        if si:
                            pend_w.extend(si.on_wait or [])
                            pend_u.extend(si.on_update or [])
                        changed = True
                        continue
                    last_sig = s_
                elif op == "Matmult":
                    pass          # matmuls don't disturb loaded weights
                elif op in ("NoOp", "EventSemaphore"):
                    pass
                else:
                    last_sig = None
                if pend_w or pend_u:
                    si = inst.sync_info
                    ow = list(si.on_wait or []) if si else []
                    ou = list(si.on_update or []) if si else []
                    inst.sync_info = mybir.SyncInfo(
                        on_wait=pend_w + ow, on_update=pend_u + ou)
                    pend_w, pend_u = [], []
                out.append(inst)
            assert not pend_w and not pend_u
            if changed:
                blk.instructions = out


def _split_excess_waits(nc):
    """The TRN2 walrus codegen allows only 1 sem-wait command per
    instruction.  Tile's sem-assigner can emit more (one per logical
    proc a tile depends on).
    Move the excess onto freshly inserted same-engine NoOps placed just
    before the instruction -- engines execute in order, so waiting on a
    preceding NoOp is equivalent."""
    import concourse.mybir as mybir
    from bass_rust import InstNoOp

    nid = [0]
    for fn in nc.m.functions:
        for blk in fn.blocks:
            out = []
            changed = False
            for inst in blk.instructions:
                si = inst.sync_info
                waits = list(si.on_wait) if si and si.on_wait else []
                limit = 1
                if len(waits) > limit:
                    extra, keep = waits[:-limit], waits[-limit:]
                    inst.sync_info = si.__replace__(on_wait=keep)
                    for w in extra:
                        nop = InstNoOp(
                            name=f"{inst.name}-wsplit{nid[0]}", ins=[], outs=[])
                        nid[0] += 1
                        nop.engine = inst.engine
                        nop.sync_info = mybir.SyncInfo(
                            on_wait=[w], on_update=[])
                        out.append(nop)
                    changed = True
                out.append(inst)
            if changed:
                blk.instructions = out


def _patch_ldw_opt():
    """walrus ships with --enable-ldw-opt=false; enabling it lets the PE
    pipeline LDWEIGHTS with in-flight matmuls (background weight buffer),
    hiding the ~100ns weight-load per matmul."""
    from concourse import bass_utils
    if getattr(bass_utils.run_command, "_ldwopt", False):
        return
    orig = bass_utils.run_command

    def run_command_ldwopt(cmd, *a, **kw):
        cmd = [c.replace("--enable-ldw-opt=false", "--enable-ldw-opt=true")
               if isinstance(c, str) else c for c in cmd]
        return orig(cmd, *a, **kw)

    run_command_ldwopt._ldwopt = True
    bass_utils.run_command = run_command_ldwopt


def _patch_act_tables():
    """Force Bacc's activation-table chooser to the single set that
    contains every function this kernel uses (exp, ln, identity, copy),
    so only one ACT_TABLE_LOAD (~2.7us each) is emitted instead of
    ping-ponging between the exp and ln sets at every softmax epilogue."""
    import concourse.hw_specs as hw_specs
    import concourse.mybir as mybir
    if getattr(hw_specs.get_activation_tables, "_attn_patched", False):
        return
    orig = hw_specs.get_activation_tables
    keep = {mybir.ActivationFunctionType.Exp, mybir.ActivationFunctionType.Ln,
            mybir.ActivationFunctionType.Identity,
            mybir.ActivationFunctionType.Copy}

    import functools

    @functools.cache
    def patched(module_arch):
        tables = dict(orig(module_arch))
        out = {}
        for name, fns in tables.items():
            if name == "natural_log_exp_and_others":
                out[name] = fns
            else:
                out[name] = fns - keep
        return out

    patched._attn_patched = True
    hw_specs.get_activation_tables = patched
    import concourse.bacc as bacc_mod
    bacc_mod.get_activation_tables = patched


def build_nc():
    """Build the per-core Bass/Tile program."""
    _ensure_paths()
    _patch_act_tables()
    if os.environ.get("ATTN_LDW_OPT", "0") == "1":
        _patch_ldw_opt()
    import concourse.bass as bass
    import concourse.mybir as mybir
    import concourse.tile as tile
    from concourse import bacc
    from contextlib import ExitStack

    f32 = mybir.dt.float32
    bf16 = mybir.dt.bfloat16

    nc = bacc.Bacc("TRN2", target_bir_lowering=False, debug=False,
                   num_devices=NCORES)

    xT_ext = nc.declare_dram_parameter("xT", [C, N], f32, isOutput=False)
    qkwT_ext = nc.declare_dram_parameter("qkwT", [C, 2 * C], f32, isOutput=False)
    qkb_ext = nc.declare_dram_parameter("qkb", [P, 2 * C // P], f32, isOutput=False)
    vwT_ext = nc.declare_dram_parameter("vwT", [C, C], f32, isOutput=False)
    vb_ext = nc.declare_dram_parameter("vb", [1, C], f32, isOutput=False)
    at_ext = nc.declare_dram_parameter(
        "at", [NPAIR, NQT, NKT, P, 2 * QW], f32, isOutput=False)
    pwT_ext = nc.declare_dram_parameter("pwT", [C, C], f32, isOutput=False)
    pb_ext = nc.declare_dram_parameter("pb", [P, C // P], f32, isOutput=False)
    out_ext = nc.declare_dram_parameter("out", [C, N], f32, isOutput=True)

    NQK = 2 * C // P   # 12 cout tiles for q|k

    with tile.TileContext(nc, num_cores=NCORES) as tc, ExitStack() as ctx:
        consts = ctx.enter_context(tc.tile_pool(name="consts", bufs=1))
        persist = ctx.enter_context(tc.tile_pool(name="persist", bufs=1))
        attn_pool = ctx.enter_context(tc.tile_pool(name="attnout", bufs=1))
        epool = ctx.enter_context(tc.tile_pool(name="epool", bufs=2))
        atp = ctx.enter_context(tc.tile_pool(name="atp", bufs=6))
        atbf = ctx.enter_context(tc.tile_pool(name="atbf", bufs=5))
        small = ctx.enter_context(tc.tile_pool(name="small", bufs=2))

        qkb_sb = consts.tile([P, NQK], f32)
        nc.sync.dma_start(qkb_sb[:], qkb_ext[:])
        pb_sb = consts.tile([P, NCIN], f32)
        nc.sync.dma_start(pb_sb[:], pb_ext[:])
        vbf_sb = consts.tile([1, C], f32)
        nc.sync.dma_start(vbf_sb[:], vb_ext[:])
        vb_sb = consts.tile([1, C], bf16)
        nc.vector.tensor_copy(vb_sb[:], vbf_sb[:])
        ones_sb = consts.tile([1, P], bf16)
        nc.any.memset(ones_sb[:], 1.0)
        ones64_sb = consts.tile([33, 64], f32)
        nc.any.memset(ones64_sb[:], 1.0)   # rows 0 and 32 used as rank-1 lhsT

        # persistent activations (bf16 matmul operands)
        # per-pair q/k tensors [P, 2 (q|k), N], written right before the
        # pair's attention work so qkT matmuls interleave with attention
        qkT_prs = [persist.tile([P, 2, N], bf16, tag=f"qkt{p}",
                                name=f"qkt{p}")
                   for p in range(NPAIR)]
        vp_sb = persist.tile([P, H, NKT, 65], bf16)   # [V_h | 1] stationary
        nc.any.memset(vp_sb[:, :, :, 64:65], 1.0)
        pw_sb = persist.tile([P, NCIN, C], bf16)      # proj weights
        attn_sb = attn_pool.tile([P, NCIN, N], bf16)  # attention out^T

        with tc.tile_pool(name="ph1", bufs=1) as ph1, \
             tc.tile_pool(name="stgp", bufs=8) as stgp:
            xT_sb = ph1.tile([P, NCIN, N], bf16)
            qkw_sb = ph1.tile([P, NCIN, 2 * C], bf16)
            vw_sb = ph1.tile([P, NCIN, C], bf16)
            # staged f32 loads (double-buffered, per-kchunk) casted into
            # bf16 tensors, so matmuls start before all weights land
            xT_r = xT_ext.rearrange("(c p) t -> p c t", p=P)
            qkw_r = qkwT_ext.rearrange("(c p) n -> p c n", p=P)
            vw_r = vwT_ext.rearrange("(c p) n -> p c n", p=P)
            pw_r = pwT_ext.rearrange("(c p) n -> p c n", p=P)
            loads = []
            for kc in range(NCIN):
                loads.append((xT_r[:, kc, :], xT_sb[:, kc, :], N))
                loads.append((vw_r[:, kc, :], vw_sb[:, kc, :], C))
            for kc in range(NCIN):
                loads.append((qkw_r[:, kc, :], qkw_sb[:, kc, :], 2 * C))
            for kc in range(NCIN):
                loads.append((pw_r[:, kc, :], pw_sb[:, kc, :], C))
            for src_ap, dst_ap, w in loads:
                for w0 in range(0, w, QW):
                    pw_ = min(QW, w - w0)
                    stg = stgp.tile([P, QW], f32, tag="stage")
                    nc.sync.dma_start(stg[:, 0:pw_], src_ap[:, w0:w0 + pw_])
                    nc.vector.tensor_copy(
                        dst_ap[:, w0:w0 + pw_], stg[:, 0:pw_])

            # ---- V (kc-outer so matmuls start with the first chunks) ----
            with tc.tile_pool(name="pp_v", bufs=2, space="PSUM") as pp_v:
                for grp in range(4):
                    tts = (2 * grp, 2 * grp + 1)
                    pss = {}
                    for tt in tts:
                        pss[tt] = pp_v.tile([P, C], f32, tag="v",
                                            name=f"vps{tt}")
                    for kc in range(NCIN):
                        for tt in tts:
                            for (n0, nw) in ((0, QW), (QW, C - QW)):
                                nc.tensor.matmul(
                                    pss[tt][:, n0:n0 + nw],
                                    xT_sb[:, kc, tt * P:(tt + 1) * P],
                                    vw_sb[:, kc, n0:n0 + nw],
                                    start=(kc == 0), stop=False,
                                    skip_group_check=True)
                    for tt in tts:
                        for (n0, nw) in ((0, QW), (QW, C - QW)):
                            nc.tensor.matmul(
                                pss[tt][:, n0:n0 + nw],
                                ones_sb[0:1, 0:P],
                                vb_sb[0:1, n0:n0 + nw],
                                start=False, stop=True,
                                skip_group_check=True)
                        nc.scalar.copy(
                            vp_sb[:, :, tt, 0:64],
                            pss[tt].rearrange("p (h d) -> p h d", d=64))

            # ---- attention (+ interleaved qkT groups) ----
            with tc.tile_pool(name="pp_st", bufs=2, space="PSUM") as pp_st, \
                 tc.tile_pool(name="pp_ev", bufs=2, space="PSUM") as pp_ev, \
                     tc.tile_pool(name="pp_av", bufs=2, space="PSUM") as pp_av:

                def qkt_group(pr):
                    for qki, ct in ((0, pr), (1, NPAIR + pr)):
                        ps = pp_st.tile([P, N], f32, tag="st",
                                        name=f"qk{ct}")
                        for kc in range(NCIN):
                            for qh in range(NQT):
                                nc.tensor.matmul(
                                    ps[:, qh * QW:(qh + 1) * QW],
                                    qkw_sb[:, kc, ct * P:(ct + 1) * P],
                                    xT_sb[:, kc, qh * QW:(qh + 1) * QW],
                                    start=(kc == 0), stop=(kc == NCIN - 1),
                                    skip_group_check=True)
                        nc.scalar.activation(
                            qkT_prs[pr][:, qki, :], ps[:, :],
                            mybir.ActivationFunctionType.Identity,
                            bias=qkb_sb[:, ct:ct + 1])

                def emit_st_step(pr, qt, e_sb, kt):
                    q0 = qt * QW
                    st = pp_st.tile([P, 2 * QW], f32, tag="st",
                                    name=f"st{pr}_{qt}_{kt}")
                    k0 = kt * P
                    nc.tensor.matmul(
                        st[:, 0:QW],
                        qkT_prs[pr][0:64, 1, k0:k0 + P],
                        qkT_prs[pr][0:64, 0, q0:q0 + QW],
                        start=True, stop=True)
                    nc.tensor.matmul(
                        st[:, QW:2 * QW],
                        qkT_prs[pr][64:128, 1, k0:k0 + P],
                        qkT_prs[pr][64:128, 0, q0:q0 + QW],
                        start=True, stop=True)
                    nc.scalar.activation(
                        e_sb[:, kt, :], st[:, :],
                        mybir.ActivationFunctionType.Exp, scale=SCALE)

                def emit_out_step(item, kt):
                    pr, qt, e_sb, psE1, psE2, psA = item
                    h1, h2 = 2 * pr, 2 * pr + 1
                    at_f = atp.tile([P, 2 * QW], f32, tag="at",
                                    name=f"atf{pr}_{qt}_{kt}")
                    nc.sync.dma_start(at_f[:], at_ext[pr, qt, kt])
                    at = atbf.tile([P, 2 * QW], bf16, tag="atb",
                                   name=f"atb{pr}_{qt}_{kt}")
                    nc.vector.tensor_copy(at[:], at_f[:])
                    st_flags = dict(start=(kt == 0), stop=(kt == NKT - 1))
                    nc.tensor.matmul(
                        psE1[0:65, :], vp_sb[:, h1, kt, :],
                        e_sb[:, kt, 0:QW], **st_flags)
                    nc.tensor.matmul(
                        psE2[0:65, :], vp_sb[:, h2, kt, :],
                        e_sb[:, kt, QW:2 * QW], **st_flags)
                    nc.tensor.matmul(
                        psA[0:64, :], vp_sb[:, h1, kt, 0:64],
                        at[:, 0:QW], **st_flags)
                    nc.tensor.matmul(
                        psA[64:128, :], vp_sb[:, h2, kt, 0:64],
                        at[:, QW:2 * QW], **st_flags)

                def emit_epilogue_act(item):
                    # reciprocal chain on ACT; runs while the next block's
                    # score matmuls keep the PE busy
                    pr, qt, e_sb, psE1, psE2, psA = item
                    rs = []
                    for hi, psE in ((0, psE1), (1, psE2)):
                        lns_sb = small.tile([1, QW], f32, tag="lns",
                                            name=f"ln{pr}_{qt}_{hi}")
                        nc.scalar.activation(
                            lns_sb[:], psE[64:65, :],
                            mybir.ActivationFunctionType.Ln)
                        r_sb = small.tile([1, QW], f32, tag="r",
                                          name=f"r{pr}_{qt}_{hi}")
                        nc.scalar.activation(
                            r_sb[:], lns_sb[:],
                            mybir.ActivationFunctionType.Exp, scale=-1.0)
                        rs.append(r_sb)
                    return rs

                def emit_epilogue_pe(item, rs):
                    pr, qt, e_sb, psE1, psE2, psA = item
                    q0 = qt * QW
                    for hi, psE in ((0, psE1), (1, psE2)):
                        pa, pz = hi * 64, hi * 64 + 64
                        nc.tensor.matmul(psE[64:128, :],
                                         ones64_sb[0:1, :],
                                         rs[hi][:, :], start=True, stop=True)
                        rb_sb = small.tile([64, QW], f32, tag="rb",
                                           name=f"rb{pr}_{qt}_{hi}")
                        nc.vector.tensor_copy(rb_sb[:], psE[64:128, :])
                        dst = attn_sb[pa:pz, pr, q0:q0 + QW]
                        nc.vector.tensor_mul(dst, psE[0:64, :], rb_sb[:])
                        nc.vector.tensor_add(dst, dst, psA[pa:pz, :])

                # software-pipelined emission: item i's ST/exp stream is
                # interleaved kt-by-kt with item i-1's E@v/A@v matmuls, so
                # the PE has dense work while ACT drains the score tiles
                items = [(pr, qt) for pr in range(NPAIR)
                         for qt in range(NQT)]
                prev = None        # item whose OUT runs in the current block
                pend = None        # (item, rs): awaiting its PE/DVE epilogue
                for pr, qt in items:
                    if qt == 0:
                        qkt_group(pr)
                    e_sb = epool.tile([P, NKT, 2 * QW], bf16, tag="e",
                                      name=f"e{pr}_{qt}")
                    # two score steps up front cover the pending epilogue's
                    # ACT reciprocal latency before its PE part is issued
                    emit_st_step(pr, qt, e_sb, 0)
                    emit_st_step(pr, qt, e_sb, 1)
                    if pend is not None:
                        emit_epilogue_pe(*pend)
                        pend = None
                    psE1 = pp_ev.tile([P, QW], f32, tag="ev",
                                      name=f"ev1_{pr}_{qt}")
                    psE2 = pp_ev.tile([P, QW], f32, tag="ev",
                                      name=f"ev2_{pr}_{qt}")
                    psA = pp_av.tile([P, QW], f32, tag="av",
                                     name=f"av{pr}_{qt}")
                    cur = (pr, qt, e_sb, psE1, psE2, psA)
                    for kt in range(NKT):
                        if kt + 2 < NKT:
                            emit_st_step(pr, qt, e_sb, kt + 2)
                        if prev is not None:
                            emit_out_step(prev, kt)
                    if prev is not None:
                        pend = (prev, emit_epilogue_act(prev))
                    prev = cur
                # drain the last item unpipelined
                for kt in range(NKT):
                    emit_out_step(prev, kt)
                if pend is not None:
                    emit_epilogue_pe(*pend)
                emit_epilogue_pe(prev, emit_epilogue_act(prev))

                # ---- output projection ----
                with tc.tile_pool(name="ph3o", bufs=2) as ph3o:
                    out_r = out_ext.rearrange("(c p) t -> p c t", p=P)
                    for ct in range(NCIN):
                        ps = pp_st.tile([P, N], f32, tag="st",
                                        name=f"proj{ct}")
                        for kc in range(NCIN):
                            for qh in range(NQT):
                                nc.tensor.matmul(
                                    ps[:, qh * QW:(qh + 1) * QW],
                                    pw_sb[:, kc, ct * P:(ct + 1) * P],
                                    attn_sb[:, kc, qh * QW:(qh + 1) * QW],
                                    start=(kc == 0), stop=(kc == NCIN - 1),
                                    skip_group_check=True)
                        o_sb = ph3o.tile([P, N], f32, tag="o",
                                         name=f"o{ct}")
                        nc.vector.tensor_scalar_add(
                            o_sb[:], ps[:], pb_sb[:, ct:ct + 1])
                        nc.sync.dma_start(out_r[:, ct, :], o_sb[:])

    if os.environ.get("ATTN_FUSE_LDW", "0") == "1":
        _fuse_ldweights(nc)
    if os.environ.get("ATTN_DEDUP_LDW", "1") == "1":
        _dedup_ldweights(nc)
    if os.environ.get("ATTN_SPLIT_WAITS", "0") == "1":
        _split_excess_waits(nc)
    if not nc.is_finalized():
        nc.finalize()   # Bacc: move_matmul_waits + generate_event_semaphores
    return nc


def make_in_maps(x, qkv_w, qkv_b, static_a, proj_w, proj_b):
    """Host-side sharding / layout prep. One batch element per core."""
    x = np.asarray(x, dtype=np.float32)
    qkv_w = np.asarray(qkv_w, dtype=np.float32)
    qkv_b = np.asarray(qkv_b, dtype=np.float32)
    static_a = np.asarray(static_a, dtype=np.float32)
    proj_w = np.asarray(proj_w, dtype=np.float32)
    proj_b = np.asarray(proj_b, dtype=np.float32)

    qkwT = np.ascontiguousarray(qkv_w[0:2 * C].T)            # [768, 1536]
    qkb = np.ascontiguousarray(qkv_b[0:2 * C].reshape(2 * C // P, P).T)
    vwT = np.ascontiguousarray(qkv_w[2 * C:3 * C].T)         # [768, 768]
    vb = np.ascontiguousarray(qkv_b[2 * C:3 * C].reshape(1, C))
    # A^T strips, contiguous per (pair, qtile, ktile): [6, 2, 8, 128, 1024]
    # at[pr, qt, kt, :, 0:512] = A^T[2pr][kt tile, qt tile], [..., 512:] = head 2pr+1
    atT = static_a[0].transpose(0, 2, 1)                      # [H, k, q]
    at = np.ascontiguousarray(
        atT.reshape(NPAIR, 2, NKT, P, NQT, QW).transpose(0, 4, 2, 3, 1, 5)
        .reshape(NPAIR, NQT, NKT, P, 2 * QW))
    pwT = np.ascontiguousarray(proj_w.T)
    pb = np.ascontiguousarray(proj_b.reshape(C // P, P).T)

    shared = {"qkwT": qkwT, "qkb": qkb, "vwT": vwT, "vb": vb,
              "at": at, "pwT": pwT, "pb": pb}
    in_maps = []
    for b in range(B):
        m = dict(shared)
        m["xT"] = np.ascontiguousarray(x[b].T)
        in_maps.append(m)
    return in_maps


_NC_CACHE = {}


def _get_nc():
    if "nc" not in _NC_CACHE:
        _NC_CACHE["nc"] = build_nc()
    return _NC_CACHE["nc"]


def kernel(x, qkv_w, qkv_b, static_a, proj_w, proj_b):
    _ensure_paths()
    from concourse.bass_utils import run_bass_kernel_spmd

    nc = _get_nc()
    in_maps = make_in_maps(x, qkv_w, qkv_b, static_a, proj_w, proj_b)
    res = run_bass_kernel_spmd(nc, in_maps, core_ids=list(range(NCORES)))
    out = np.empty((B, N, C), dtype=np.float32)
    for b in range(B):
        out[b] = res.results[b]["out"].T
    return out
